# revision 3
# baseline (speedup 1.0000x reference)
"""Trainium2 Bass kernel for nn_Attention_41343355191713 (GNN message-passing
attention). Single SPMD launch on 8 cores:

  P1: QKV projection on each core's 8192-node slice (Q pre-scaled), K/V/Q
      stashed in SBUF, Q slice AllGathered to every core.
  P2: edges sorted by j (host), sharded so core c owns all edges whose j lands
      in its node range. Per 128-node window: gather Q[i] rows (indirect DMA),
      K[j] via one-hot matmul from the SBUF stash, A = Q.K per head,
      exp(A - 8), segment-sum into denom via one-hot matmul (softmax without
      max-subtraction: |A| <= ~7 for this distribution, and a constant shift
      cancels exactly). V normalized by denom, K|Vn AllGathered.
  P3: edges sorted by i; per destination window gather K|Vn[j] rows, recompute
      A, w = exp(A-8)*Vn, segment-sum into attn via one-hot matmul, then the
      fused epilogue (residual + LN + silu MLP + LN) and fp16 store.

Indices ship as uint16/uint8 (6B/edge), h_one and the output as fp16 — the
axon tunnel (~70MB/s) dominates cost, so bytes moved is the metric.
"""

import sys

sys.path.insert(0, "/opt/trn_rl_repo")

import math

import numpy as np

import concourse.bass as bass
import concourse.bacc as bacc
import concourse.mybir as mybir
import concourse.tile as tile
from concourse.bass import ds
from concourse.bass_utils import run_bass_kernel_spmd
from concourse.masks import make_identity

N = 65536
DIM = 128
HEADS = 4
HD = DIM // HEADS
SCALE = 1.0 / math.sqrt(HD)
LN_EPS = 1e-6
NCORES = 8
P = 128
SLICE = N // NCORES          # 8192 nodes per core
WPC = SLICE // P             # 64 windows per core
NW = N // P                  # 512 windows global
DEFAULT_TMAX = 34            # padded 128-edge tiles per window
ECONST = 8.0                 # constant shift inside exp
F32 = mybir.dt.float32
F16 = mybir.dt.float16
BF16 = mybir.dt.bfloat16
I32 = mybir.dt.int32
U16 = mybir.dt.uint16
U8 = mybir.dt.uint8

_cache = {}


def _build(TMAX):
    nc = bacc.Bacc(None, target_bir_lowering=False, num_devices=NCORES)
    h_sl = nc.declare_dram_parameter("h_sl", [SLICE, DIM], F16, isOutput=False)
    wqm = nc.declare_dram_parameter("wqm", [DIM, 4 * DIM], F32, isOutput=False)
    # index packs (u8): per-window blocks of 3T cols [lo T | hi T | loc T]
    idxpack2 = nc.declare_dram_parameter("idxpack2", [P, 3 * WPC * TMAX], U8,
                                         isOutput=False)
    idxpack3 = nc.declare_dram_parameter("idxpack3", [P, 3 * WPC * TMAX], U8,
                                         isOutput=False)
    out = nc.declare_dram_parameter("out", [SLICE, DIM], F16, isOutput=True)

    qsl_d = nc.dram_tensor("qsl_d", [SLICE, DIM], F32, kind="Internal")
    kvn_d = nc.dram_tensor("kvn_d", [SLICE, 2 * DIM], F32, kind="Internal")
    qfull = nc.dram_tensor("qfull", [N, DIM], F32, kind="Internal")
    kvnfull = nc.dram_tensor("kvnfull", [N, 2 * DIM], F32, kind="Internal")

    with tile.TileContext(nc) as tc:
        with (
            tc.tile_pool(name="const", bufs=1) as cpool,
            tc.tile_pool(name="stash", bufs=1) as spool,
            tc.tile_pool(name="work", bufs=3) as wpool,
            tc.tile_pool(name="gath", bufs=4) as gpool,
        ):
            # ---- constants ----
            ident = cpool.tile([P, P], F32)
            make_identity(nc, ident[:])
            ident_b = cpool.tile([P, P], BF16)
            nc.vector.tensor_copy(out=ident_b[:], in_=ident[:])
            iota_i = cpool.tile([P, P], I32)
            nc.gpsimd.iota(iota_i[:], pattern=[[1, P]], base=0, channel_multiplier=0)
            iotaPQ = cpool.tile([P, P], F32)
            nc.vector.tensor_copy(out=iotaPQ[:], in_=iota_i[:])
            negc = cpool.tile([P, 1], F32)
            nc.gpsimd.memset(negc[:], -ECONST)
            eps_t = cpool.tile([P, 1], F32)
            nc.gpsimd.memset(eps_t[:], LN_EPS)

            wq_f = cpool.tile([P, 3 * DIM], F32)
            nc.sync.dma_start(out=wq_f[:], in_=wqm[:, 0:3 * DIM])
            wq_b = cpool.tile([P, 3 * DIM], BF16)
            nc.vector.tensor_copy(out=wq_b[:], in_=wq_f[:])
            wq_r = cpool.tile([P, 3 * DIM], BF16)
            nc.vector.tensor_tensor(out=wq_r[:], in0=wq_f[:], in1=wq_b[:],
                                    op=mybir.AluOpType.subtract)
            wm_f = cpool.tile([P, DIM], F32)
            nc.sync.dma_start(out=wm_f[:], in_=wqm[:, 3 * DIM:4 * DIM])
            wm_b = cpool.tile([P, DIM], BF16)
            nc.vector.tensor_copy(out=wm_b[:], in_=wm_f[:])
            wm_r = cpool.tile([P, DIM], BF16)
            nc.vector.tensor_tensor(out=wm_r[:], in0=wm_f[:], in1=wm_b[:],
                                    op=mybir.AluOpType.subtract)

            # ---- persistent stashes ----
            qb_st = spool.tile([P, WPC * P], BF16)     # Q (scaled) per window
            kb_st = spool.tile([P, WPC * P], BF16)     # K per window
            vf_st = spool.tile([P, WPC * P], F32)      # V per window
            sel_all = spool.tile([P, TMAX * P], BF16)  # per-window one-hots
            msg_all = spool.tile([P, TMAX * P], F32)
            msgb_all = spool.tile([P, TMAX * P], BF16)
            expa_all = spool.tile([P, TMAX * HEADS], F32)
            expab_all = spool.tile([P, TMAX * HEADS], BF16)

            # ================= P1: QKV projection =================
            ps1_cm = tc.tile_pool(name="ps1", bufs=2, space="PSUM")
            pspool = ps1_cm.__enter__()
            for t in range(WPC):
                ht = wpool.tile([P, P], F16, tag="ht")
                nc.sync.dma_start(out=ht[:], in_=h_sl[t * P:(t + 1) * P, :])
                h32 = wpool.tile([P, P], F32, tag="h32")
                nc.vector.tensor_copy(out=h32[:], in_=ht[:])
                hb = wpool.tile([P, P], BF16, tag="hb")
                nc.vector.tensor_copy(out=hb[:], in_=h32[:])
                hr = wpool.tile([P, P], BF16, tag="hr")
                nc.vector.tensor_tensor(out=hr[:], in0=h32[:], in1=hb[:],
                                        op=mybir.AluOpType.subtract)
                hbT_ps = pspool.tile([P, P], BF16, tag="tp")
                nc.tensor.transpose(out=hbT_ps[:], in_=hb[:], identity=ident_b[:])
                hbT = wpool.tile([P, P], BF16, tag="hbT")
                nc.scalar.copy(out=hbT[:], in_=hbT_ps[:])
                hrT_ps = pspool.tile([P, P], BF16, tag="tpr")
                nc.tensor.transpose(out=hrT_ps[:], in_=hr[:], identity=ident_b[:])
                hrT = wpool.tile([P, P], BF16, tag="hrT")
                nc.scalar.copy(out=hrT[:], in_=hrT_ps[:])
                o_ps = pspool.tile([P, 3 * DIM], F32, tag="o")
                nc.tensor.matmul(out=o_ps[:], lhsT=hbT[:], rhs=wq_b[:],
                                 start=True, stop=False)
                nc.tensor.matmul(out=o_ps[:], lhsT=hrT[:], rhs=wq_b[:],
                                 start=False, stop=False)
                nc.tensor.matmul(out=o_ps[:], lhsT=hbT[:], rhs=wq_r[:],
                                 start=False, stop=True)
                qs = wpool.tile([P, DIM], F32, tag="qs")
                nc.scalar.copy(out=qs[:], in_=o_ps[:, 0:DIM])
                nc.sync.dma_start(out=qsl_d[t * P:(t + 1) * P, :], in_=qs[:])
                nc.vector.tensor_copy(out=qb_st[:, t * P:(t + 1) * P],
                                      in_=o_ps[:, 0:DIM])
                nc.vector.tensor_copy(out=kb_st[:, t * P:(t + 1) * P],
                                      in_=o_ps[:, DIM:2 * DIM])
                nc.vector.tensor_copy(out=vf_st[:, t * P:(t + 1) * P],
                                      in_=o_ps[:, 2 * DIM:3 * DIM])

            ps1_cm.__exit__(None, None, None)
            nc.gpsimd.collective_compute(
                "AllGather", mybir.AluOpType.bypass,
                replica_groups=[list(range(NCORES))],
                ins=[qsl_d[:].opt()], outs=[qfull[:].opt()])
            ps2_cm = tc.tile_pool(name="ps2", bufs=2, space="PSUM")
            pspool = ps2_cm.__enter__()
            acc2_cm = tc.tile_pool(name="acc2", bufs=1, space="PSUM")
            accpool = acc2_cm.__enter__()

            # ================= P2: denominators =================
            with tc.For_i(0, WPC, 1) as w:
                blk2 = wpool.tile([P, 3 * TMAX], U8, tag="blk2")
                nc.sync.dma_start(out=blk2[:],
                                  in_=idxpack2[:, ds(w * 3 * TMAX, 3 * TMAX)])
                lo32 = wpool.tile([P, TMAX], I32, tag="lo32")
                nc.vector.tensor_copy(out=lo32[:], in_=blk2[:, 0:TMAX])
                hi32 = wpool.tile([P, TMAX], I32, tag="hi32")
                nc.vector.tensor_copy(out=hi32[:], in_=blk2[:, TMAX:2 * TMAX])
                hs32 = wpool.tile([P, TMAX], I32, tag="hs32")
                nc.vector.tensor_scalar_mul(hs32[:], hi32[:], 256)
                iblk = wpool.tile([P, TMAX], I32, tag="iblk")
                nc.vector.tensor_tensor(out=iblk[:], in0=hs32[:], in1=lo32[:],
                                        op=mybir.AluOpType.add)
                jlf = wpool.tile([P, TMAX], F32, tag="jlf")
                nc.vector.tensor_copy(out=jlf[:], in_=blk2[:, 2 * TMAX:3 * TMAX])

                for t in range(TMAX):
                    qe = gpool.tile([P, DIM], F32, tag="qe")
                    nc.gpsimd.indirect_dma_start(
                        out=qe[:], out_offset=None, in_=qfull[:],
                        in_offset=bass.IndirectOffsetOnAxis(
                            ap=iblk[:, t:t + 1], axis=0))
                    nc.vector.tensor_tensor(
                        out=sel_all[:, t * P:(t + 1) * P],
                        in0=jlf[:, t:t + 1].to_broadcast([P, P]), in1=iotaPQ[:],
                        op=mybir.AluOpType.is_equal)
                    selT_ps = pspool.tile([P, P], BF16, tag="selT")
                    nc.tensor.transpose(out=selT_ps[:],
                                        in_=sel_all[:, t * P:(t + 1) * P],
                                        identity=ident_b[:])
                    selT = wpool.tile([P, P], BF16, tag="selTs")
                    nc.scalar.copy(out=selT[:], in_=selT_ps[:])
                    ke_ps = pspool.tile([P, DIM], F32, tag="ke")
                    nc.tensor.matmul(out=ke_ps[:], lhsT=selT[:],
                                     rhs=kb_st[:, ds(w * P, P)],
                                     start=True, stop=True)
                    prod = wpool.tile([P, DIM], F32, tag="prod")
                    nc.vector.tensor_tensor(out=prod[:], in0=qe[:], in1=ke_ps[:],
                                            op=mybir.AluOpType.mult)
                    a_t = wpool.tile([P, HEADS], F32, tag="a_t")
                    nc.vector.tensor_reduce(
                        out=a_t[:], in_=prod[:].rearrange("p (h d) -> p h d", h=HEADS),
                        axis=mybir.AxisListType.X, op=mybir.AluOpType.add)
                    nc.scalar.activation(
                        out=expa_all[:, t * HEADS:(t + 1) * HEADS], in_=a_t[:],
                        func=mybir.ActivationFunctionType.Exp,
                        bias=negc[:, 0:1], scale=1.0)

                nc.vector.tensor_copy(out=expab_all[:], in_=expa_all[:])
                den_ps = accpool.tile([P, HEADS], F32, tag="den")
                for t in range(TMAX):
                    nc.tensor.matmul(
                        out=den_ps[:], lhsT=sel_all[:, t * P:(t + 1) * P],
                        rhs=expab_all[:, t * HEADS:(t + 1) * HEADS],
                        start=(t == 0), stop=(t == TMAX - 1))
                den_s = wpool.tile([P, HEADS], F32, tag="den_s")
                nc.vector.tensor_scalar_add(den_s[:], den_ps[:], 1e-20)
                rec = wpool.tile([P, HEADS], F32, tag="rec")
                nc.vector.reciprocal(out=rec[:], in_=den_s[:])
                kf = wpool.tile([P, DIM], F32, tag="kf")
                nc.vector.tensor_copy(out=kf[:], in_=kb_st[:, ds(w * P, P)])
                nc.sync.dma_start(out=kvn_d[ds(w * P, P), 0:DIM], in_=kf[:])
                vn = wpool.tile([P, DIM], F32, tag="vn")
                for h in range(HEADS):
                    nc.vector.tensor_scalar_mul(
                        vn[:, h * HD:(h + 1) * HD],
                        vf_st[:, ds(w * P + h * HD, HD)], rec[:, h:h + 1])
                nc.sync.dma_start(out=kvn_d[ds(w * P, P), DIM:2 * DIM], in_=vn[:])

            acc2_cm.__exit__(None, None, None)
            ps2_cm.__exit__(None, None, None)
            nc.gpsimd.collective_compute(
                "AllGather", mybir.AluOpType.bypass,
                replica_groups=[list(range(NCORES))],
                ins=[kvn_d[:].opt()], outs=[kvnfull[:].opt()])
            ps3_cm = tc.tile_pool(name="ps3", bufs=2, space="PSUM")
            pspool = ps3_cm.__enter__()
            acc3_cm = tc.tile_pool(name="acc3", bufs=1, space="PSUM")
            accpool = acc3_cm.__enter__()

            # ================= P3: attention + epilogue =================
            def layer_norm(src, tag):
                mu = wpool.tile([P, 1], F32, tag=f"{tag}mu")
                nc.vector.tensor_reduce(out=mu[:], in_=src,
                                        axis=mybir.AxisListType.X,
                                        op=mybir.AluOpType.add)
                mus = wpool.tile([P, 1], F32, tag=f"{tag}mus")
                nc.vector.tensor_scalar_mul(mus[:], mu[:], 1.0 / DIM)
                cen = wpool.tile([P, DIM], F32, tag=f"{tag}cen")
                nc.vector.tensor_scalar(out=cen[:], in0=src, scalar1=mus[:, 0:1],
                                        scalar2=None, op0=mybir.AluOpType.subtract)
                sq = wpool.tile([P, DIM], F32, tag=f"{tag}sq")
                vs = wpool.tile([P, 1], F32, tag=f"{tag}vs")
                nc.scalar.activation(out=sq[:], in_=cen[:],
                                     func=mybir.ActivationFunctionType.Square,
                                     accum_out=vs[:])
                sd = wpool.tile([P, 1], F32, tag=f"{tag}sd")
                nc.scalar.activation(out=sd[:], in_=vs[:],
                                     func=mybir.ActivationFunctionType.Sqrt,
                                     scale=1.0 / DIM, bias=eps_t[:, 0:1])
                rstd = wpool.tile([P, 1], F32, tag=f"{tag}rstd")
                nc.vector.reciprocal(out=rstd[:], in_=sd[:])
                o = wpool.tile([P, DIM], F32, tag=f"{tag}o")
                nc.vector.tensor_scalar_mul(o[:], cen[:], rstd[:, 0:1])
                return o

            with tc.For_i(0, WPC, 1) as w:
                blk3 = wpool.tile([P, 3 * TMAX], U8, tag="blk3")
                nc.sync.dma_start(
                    out=blk3[:], in_=idxpack3[:, ds(w * 3 * TMAX, 3 * TMAX)])
                lo33 = wpool.tile([P, TMAX], I32, tag="lo33")
                nc.vector.tensor_copy(out=lo33[:], in_=blk3[:, 0:TMAX])
                hi33 = wpool.tile([P, TMAX], I32, tag="hi33")
                nc.vector.tensor_copy(out=hi33[:], in_=blk3[:, TMAX:2 * TMAX])
                hs33 = wpool.tile([P, TMAX], I32, tag="hs33")
                nc.vector.tensor_scalar_mul(hs33[:], hi33[:], 256)
                jblk = wpool.tile([P, TMAX], I32, tag="jblk")
                nc.vector.tensor_tensor(out=jblk[:], in0=hs33[:], in1=lo33[:],
                                        op=mybir.AluOpType.add)
                ilf = wpool.tile([P, TMAX], F32, tag="ilf")
                nc.vector.tensor_copy(out=ilf[:], in_=blk3[:, 2 * TMAX:3 * TMAX])

                for t in range(TMAX):
                    kve = gpool.tile([P, 2 * DIM], F32, tag="kve")
                    nc.gpsimd.indirect_dma_start(
                        out=kve[:], out_offset=None, in_=kvnfull[:],
                        in_offset=bass.IndirectOffsetOnAxis(
                            ap=jblk[:, t:t + 1], axis=0))
                    nc.vector.tensor_tensor(
                        out=sel_all[:, t * P:(t + 1) * P],
                        in0=ilf[:, t:t + 1].to_broadcast([P, P]), in1=iotaPQ[:],
                        op=mybir.AluOpType.is_equal)
                    selT_ps = pspool.tile([P, P], BF16, tag="selT")
                    nc.tensor.transpose(out=selT_ps[:],
                                        in_=sel_all[:, t * P:(t + 1) * P],
                                        identity=ident_b[:])
                    selT = wpool.tile([P, P], BF16, tag="selTs")
                    nc.scalar.copy(out=selT[:], in_=selT_ps[:])
                    qe_ps = pspool.tile([P, DIM], F32, tag="qeps")
                    nc.tensor.matmul(out=qe_ps[:], lhsT=selT[:],
                                     rhs=qb_st[:, ds(w * P, P)],
                                     start=True, stop=True)
                    prod = wpool.tile([P, DIM], F32, tag="prod3")
                    nc.vector.tensor_tensor(out=prod[:], in0=qe_ps[:],
                                            in1=kve[:, 0:DIM],
                                            op=mybir.AluOpType.mult)
                    a_t = wpool.tile([P, HEADS], F32, tag="a_t3")
                    nc.vector.tensor_reduce(
                        out=a_t[:], in_=prod[:].rearrange("p (h d) -> p h d", h=HEADS),
                        axis=mybir.AxisListType.X, op=mybir.AluOpType.add)
                    expa = wpool.tile([P, HEADS], F32, tag="expa3")
                    nc.scalar.activation(out=expa[:], in_=a_t[:],
                                         func=mybir.ActivationFunctionType.Exp,
                                         bias=negc[:, 0:1], scale=1.0)
                    for h in range(HEADS):
                        nc.vector.tensor_scalar_mul(
                            msg_all[:, t * P + h * HD:t * P + (h + 1) * HD],
                            kve[:, DIM + h * HD:DIM + (h + 1) * HD],
                            expa[:, h:h + 1])

                nc.vector.tensor_copy(out=msgb_all[:], in_=msg_all[:])
                attn_ps = accpool.tile([P, DIM], F32, tag="attn")
                for t in range(TMAX):
                    nc.tensor.matmul(
                        out=attn_ps[:], lhsT=sel_all[:, t * P:(t + 1) * P],
                        rhs=msgb_all[:, t * P:(t + 1) * P],
                        start=(t == 0), stop=(t == TMAX - 1))

                # epilogue: h = LN1(h_one + attn); out = LN2(h + silu(h @ wm))
                h16w = wpool.tile([P, P], F16, tag="h16w")
                nc.sync.dma_start(out=h16w[:], in_=h_sl[ds(w * P, P), :])
                h32w = wpool.tile([P, P], F32, tag="h32w")
                nc.vector.tensor_copy(out=h32w[:], in_=h16w[:])
                h0 = wpool.tile([P, DIM], F32, tag="h0")
                nc.vector.tensor_tensor(out=h0[:], in0=attn_ps[:], in1=h32w[:],
                                        op=mybir.AluOpType.add)
                ln1 = layer_norm(h0[:], "l1")
                lnb = wpool.tile([P, P], BF16, tag="lnb")
                nc.vector.tensor_copy(out=lnb[:], in_=ln1[:])
                lt_ps = accpool.tile([P, P], BF16, tag="lt")
                nc.tensor.transpose(out=lt_ps[:], in_=lnb[:], identity=ident_b[:])
                lt = wpool.tile([P, P], BF16, tag="lt_s")
                nc.scalar.copy(out=lt[:], in_=lt_ps[:])
                y_ps = accpool.tile([P, DIM], F32, tag="y")
                nc.tensor.matmul(out=y_ps[:], lhsT=lt[:], rhs=wm_b[:],
                                 start=True, stop=False)
                nc.tensor.matmul(out=y_ps[:], lhsT=lt[:], rhs=wm_r[:],
                                 start=False, stop=True)
                y = wpool.tile([P, DIM], F32, tag="ysb")
                nc.scalar.activation(out=y[:], in_=y_ps[:],
                                     func=mybir.ActivationFunctionType.Silu)
                h2 = wpool.tile([P, DIM], F32, tag="h2")
                nc.vector.tensor_tensor(out=h2[:], in0=ln1[:], in1=y[:],
                                        op=mybir.AluOpType.add)
                ln2 = layer_norm(h2[:], "l2")
                o16 = wpool.tile([P, DIM], F16, tag="o16")
                nc.vector.tensor_copy(out=o16[:], in_=ln2[:])
                nc.sync.dma_start(out=out[ds(w * P, P), :], in_=o16[:])
            acc3_cm.__exit__(None, None, None)
            ps3_cm.__exit__(None, None, None)
    nc.compile()
    return nc


_arange_cache = {}


def _build_phase(key_arr, other_arr, E, TMAX):
    """Group edges by 128-node window of key; pad windows to TMAX*128 slots.
    Returns (other, loc) as [NCORES*128, WPC*TMAX] uint16/uint8 arrays laid
    out so column w*TMAX+t, partition p holds edge slot t*128+p of window w."""
    wid16 = (np.asarray(key_arr) >> 7).astype(np.uint16)
    order = np.argsort(wid16, kind="stable")     # radix: groups by window
    wid = wid16[order].astype(np.int64)
    cnt = np.bincount(wid, minlength=NW)
    if cnt.max() > TMAX * P:
        raise _WindowOverflow(int(cnt.max()))
    starts = np.zeros(NW, np.int64)
    np.cumsum(cnt[:-1], out=starts[1:])
    if E not in _arange_cache:
        _arange_cache[E] = np.arange(E, dtype=np.int64)
    dest = wid * np.int64(TMAX * P) + (_arange_cache[E] - starts[wid])
    oth = np.zeros(NW * TMAX * P, np.uint16)
    loc = np.full(NW * TMAX * P, 255, np.uint8)
    oth[dest] = other_arr[order].astype(np.uint16)
    loc[dest] = (key_arr[order] & 127).astype(np.uint8)
    oth = oth.reshape(NCORES, WPC, TMAX, P).transpose(0, 3, 1, 2).reshape(
        NCORES * P, WPC * TMAX)
    loc = loc.reshape(NCORES, WPC, TMAX, P).transpose(0, 3, 1, 2).reshape(
        NCORES * P, WPC * TMAX)
    return np.ascontiguousarray(oth), np.ascontiguousarray(loc)


def _pack_phase(oth, loc, TMAX):
    """[NCORES*P, C] u16 + u8 -> [NCORES*P, 3C] u8 with per-window interleave
    [lo T | hi T | loc T]."""
    R, C = oth.shape
    lo = (oth & 0xFF).astype(np.uint8).reshape(R, WPC, TMAX)
    hi = (oth >> 8).astype(np.uint8).reshape(R, WPC, TMAX)
    lc = loc.reshape(R, WPC, TMAX)
    return np.stack([lo, hi, lc], axis=2).reshape(R, 3 * C)


class _WindowOverflow(RuntimeError):
    def __init__(self, count):
        super().__init__(f"window edge count {count} exceeds padded capacity")
        self.count = count


def _get_launcher(nc, key):
    """jit-compiled single-launch dispatcher. Unlike run_bass_via_pjrt it
    creates the donated output buffers on-device (nothing shipped for them)
    and caches the compiled executable for repeat calls."""
    if ("launcher", key) in _cache:
        return _cache[("launcher", key)]
    import jax
    import jax.numpy as jnp
    from jax.experimental.shard_map import shard_map
    from jax.sharding import Mesh, NamedSharding, PartitionSpec
    from concourse import bass2jax, mybir as _mybir

    bass2jax.install_neuronx_cc_hook()
    partition_name = nc.partition_id_tensor.name if nc.partition_id_tensor else None
    in_names, out_names, out_avals = [], [], []
    for alloc in nc.m.functions[0].allocations:
        if not isinstance(alloc, _mybir.MemoryLocationSet):
            continue
        name = alloc.memorylocations[0].name
        if alloc.kind == "ExternalInput":
            if name != partition_name:
                in_names.append(name)
        elif alloc.kind == "ExternalOutput":
            shape = tuple(alloc.tensor_shape)
            out_avals.append(jax.core.ShapedArray(shape, _mybir.dt.np(alloc.dtype)))
            out_names.append(name)
    n_params = len(in_names)
    all_names = in_names + out_names + ([partition_name] if partition_name else [])

    def _body(*args):
        operands = list(args)
        if partition_name is not None:
            operands.append(bass2jax.partition_id_tensor())
        outs = bass2jax._bass_exec_p.bind(
            *operands,
            out_avals=tuple(out_avals),
            in_names=tuple(all_names),
            out_names=tuple(out_names),
            lowering_input_output_aliases=(),
            sim_require_finite=True,
            sim_require_nnan=True,
            nc=nc,
        )
        return tuple(outs)

    devices = jax.devices()[:NCORES]
    mesh = Mesh(np.asarray(devices), ("core",))
    sharding = NamedSharding(mesh, PartitionSpec("core"))
    n_outs = len(out_avals)
    donate = tuple(range(n_params, n_params + n_outs))
    sharded = jax.jit(
        shard_map(_body, mesh=mesh,
                  in_specs=(PartitionSpec("core"),) * (n_params + n_outs),
                  out_specs=(PartitionSpec("core"),) * n_outs,
                  check_rep=False),
        donate_argnums=donate, keep_unused=True)

    def make_zeros():
        return [
            jax.jit(lambda a=a: jnp.zeros((NCORES * a.shape[0],) + a.shape[1:],
                                          a.dtype), out_shardings=sharding)()
            for a in out_avals
        ]

    launcher = dict(call=sharded, in_names=in_names, out_names=out_names,
                    make_zeros=make_zeros, sharding=sharding)
    _cache[("launcher", key)] = launcher
    return launcher


def kernel(**inputs):
    import os
    import threading
    import time

    import jax
    tlog = []
    _t0 = time.time()

    def _tick(label):
        tlog.append((label, time.time() - _t0))

    h_one = np.asarray(inputs["h_one"], np.float32)
    w_qkv = np.asarray(inputs["W_qkv"], np.float32)
    w_mlp = np.asarray(inputs["W_mlp"], np.float32)
    i_arr = np.asarray(inputs["e_e_i"]).astype(np.int64)
    j_arr = np.asarray(inputs["e_e_j"]).astype(np.int64)
    E = len(i_arr)

    tmax = _cache.get("tmax", DEFAULT_TMAX)

    # stage the index prep so phase-2 arrays upload while phase-3 prep runs
    prep = {}

    def _prep2():
        try:
            oth, loc = _build_phase(j_arr, i_arr, E, tmax)
            prep["pack2"] = _pack_phase(oth, loc, tmax)
        except _WindowOverflow as e:
            prep["overflow2"] = e.count

    def _prep3():
        try:
            oth, loc = _build_phase(i_arr, j_arr, E, tmax)
            prep["pack3"] = _pack_phase(oth, loc, tmax)
        except _WindowOverflow as e:
            prep["overflow3"] = e.count

    th2 = threading.Thread(target=_prep2)
    th2.start()
    _tick("thread started")

    h16 = h_one.astype(np.float16)
    wq_scaled = w_qkv.copy()
    wq_scaled[:, :DIM] *= np.float32(SCALE)
    wqm_rep = np.tile(np.concatenate([wq_scaled, w_mlp], axis=1), (NCORES, 1))

    _tick("casts done")
    if ("nc", tmax) not in _cache:
        _cache[("nc", tmax)] = _build(tmax)
    _tick("build done")
    L = _get_launcher(_cache[("nc", tmax)], tmax)
    sh = L["sharding"]
    _tick("launcher ready")

    # start big uploads while the index prep thread still runs
    dev = {"h_sl": jax.device_put(h16, sh), "wqm": jax.device_put(wqm_rep, sh)}
    _tick("h/w device_put issued")
    th2.join()
    th3 = threading.Thread(target=_prep3)
    th3.start()
    _tick("prep2 joined")
    if "pack2" in prep:
        dev["idxpack2"] = jax.device_put(prep["pack2"], sh)
    _tick("idxpack2 device_put issued")
    th3.join()
    _tick("prep3 joined")

    over = max(prep.get("overflow2", 0), prep.get("overflow3", 0))
    if over:
        # rare fallback: a window exceeds tmax*128 edges — rebuild the
        # program with enough headroom and redo the prep
        tmax = -(-over // P) + 2
        _cache["tmax"] = tmax
        if ("nc", tmax) not in _cache:
            _cache[("nc", tmax)] = _build(tmax)
        L = _get_launcher(_cache[("nc", tmax)], tmax)
        sh = L["sharding"]
        prep.clear()
        _prep2()
        _prep3()
        dev["idxpack2"] = jax.device_put(prep["pack2"], sh)

    dev["idxpack3"] = jax.device_put(prep["pack3"], sh)
    _tick("idxpack3 device_put issued")
    zeros = L["make_zeros"]()
    _tick("zeros made")

    out_arrs = L["call"](*[dev[n] for n in L["in_names"]], *zeros)
    _tick("call returned")
    out16 = np.asarray(out_arrs[0])
    _tick("output fetched")
    if os.environ.get("KERNEL_TIMING"):
        prev = 0.0
        for label, t in tlog:
            print(f"  [{t:6.2f}s +{t-prev:5.2f}] {label}", flush=True)
            prev = t
    return out16.astype(np.float32)


# revision 4
# speedup vs baseline: 1.0910x; 1.0910x over previous
"""Trainium2 Bass kernel for nn_Attention_41343355191713 (GNN message-passing
attention). Single SPMD launch on 8 cores:

  P1: QKV projection on each core's 8192-node slice (Q pre-scaled), K/V/Q
      stashed in SBUF, Q slice AllGathered to every core.
  P2: edges sorted by j (host), sharded so core c owns all edges whose j lands
      in its node range. Per 128-node window: gather Q[i] rows (indirect DMA),
      K[j] via one-hot matmul from the SBUF stash, A = Q.K per head,
      exp(A - 8), segment-sum into denom via one-hot matmul (softmax without
      max-subtraction: |A| <= ~7 for this distribution, and a constant shift
      cancels exactly). V normalized by denom, K|Vn AllGathered.
  P3: edges sorted by i; per destination window gather K|Vn[j] rows, recompute
      A, w = exp(A-8)*Vn, segment-sum into attn via one-hot matmul, then the
      fused epilogue (residual + LN + silu MLP + LN) and fp16 store.

Indices ship as uint16/uint8 (6B/edge), h_one and the output as fp16 — the
axon tunnel (~70MB/s) dominates cost, so bytes moved is the metric.
"""

import sys

sys.path.insert(0, "/opt/trn_rl_repo")

import math

import numpy as np

import concourse.bass as bass
import concourse.bacc as bacc
import concourse.mybir as mybir
import concourse.tile as tile
from concourse.bass import ds
from concourse.bass_utils import run_bass_kernel_spmd
from concourse.masks import make_identity

N = 65536
DIM = 128
HEADS = 4
HD = DIM // HEADS
SCALE = 1.0 / math.sqrt(HD)
LN_EPS = 1e-6
NCORES = 8
P = 128
SLICE = N // NCORES          # 8192 nodes per core
WPC = SLICE // P             # 64 windows per core
NW = N // P                  # 512 windows global
DEFAULT_TMAX = 34            # padded 128-edge tiles per window
ECONST = 8.0                 # constant shift inside exp
F32 = mybir.dt.float32
F16 = mybir.dt.float16
BF16 = mybir.dt.bfloat16
I32 = mybir.dt.int32
U16 = mybir.dt.uint16
U8 = mybir.dt.uint8

_cache = {}


def _build(TMAX):
    nc = bacc.Bacc(None, target_bir_lowering=False, num_devices=NCORES,
                   disable_frame_to_traceback=True)
    h_sl = nc.declare_dram_parameter("h_sl", [SLICE, DIM], F16, isOutput=False)
    wqm = nc.declare_dram_parameter("wqm", [DIM, 4 * DIM], F32, isOutput=False)
    # index packs (u8): per-window blocks of 3T cols [lo T | hi T | loc T]
    idxpack2 = nc.declare_dram_parameter("idxpack2", [P, 3 * WPC * TMAX], U8,
                                         isOutput=False)
    idxpack3 = nc.declare_dram_parameter("idxpack3", [P, 3 * WPC * TMAX], U8,
                                         isOutput=False)
    out = nc.declare_dram_parameter("out", [SLICE, DIM], F16, isOutput=True)

    qsl_d = nc.dram_tensor("qsl_d", [SLICE, DIM], F32, kind="Internal")
    kvn_d = nc.dram_tensor("kvn_d", [SLICE, 2 * DIM], F32, kind="Internal")
    qfull = nc.dram_tensor("qfull", [N, DIM], F32, kind="Internal")
    kvnfull = nc.dram_tensor("kvnfull", [N, 2 * DIM], F32, kind="Internal")

    with tile.TileContext(nc) as tc:
        with (
            tc.tile_pool(name="const", bufs=1) as cpool,
            tc.tile_pool(name="stash", bufs=1) as spool,
            tc.tile_pool(name="work", bufs=3) as wpool,
            tc.tile_pool(name="gath", bufs=4) as gpool,
        ):
            # ---- constants ----
            ident = cpool.tile([P, P], F32)
            make_identity(nc, ident[:])
            ident_b = cpool.tile([P, P], BF16)
            nc.vector.tensor_copy(out=ident_b[:], in_=ident[:])
            iota_i = cpool.tile([P, P], I32)
            nc.gpsimd.iota(iota_i[:], pattern=[[1, P]], base=0, channel_multiplier=0)
            iotaPQ = cpool.tile([P, P], F32)
            nc.vector.tensor_copy(out=iotaPQ[:], in_=iota_i[:])
            negc = cpool.tile([P, 1], F32)
            nc.gpsimd.memset(negc[:], -ECONST)
            eps_t = cpool.tile([P, 1], F32)
            nc.gpsimd.memset(eps_t[:], LN_EPS)

            wq_f = cpool.tile([P, 3 * DIM], F32)
            nc.sync.dma_start(out=wq_f[:], in_=wqm[:, 0:3 * DIM])
            wq_b = cpool.tile([P, 3 * DIM], BF16)
            nc.vector.tensor_copy(out=wq_b[:], in_=wq_f[:])
            wq_r = cpool.tile([P, 3 * DIM], BF16)
            nc.vector.tensor_tensor(out=wq_r[:], in0=wq_f[:], in1=wq_b[:],
                                    op=mybir.AluOpType.subtract)
            wm_f = cpool.tile([P, DIM], F32)
            nc.sync.dma_start(out=wm_f[:], in_=wqm[:, 3 * DIM:4 * DIM])
            wm_b = cpool.tile([P, DIM], BF16)
            nc.vector.tensor_copy(out=wm_b[:], in_=wm_f[:])
            wm_r = cpool.tile([P, DIM], BF16)
            nc.vector.tensor_tensor(out=wm_r[:], in0=wm_f[:], in1=wm_b[:],
                                    op=mybir.AluOpType.subtract)

            # ---- persistent stashes ----
            qb_st = spool.tile([P, WPC * P], BF16)     # Q (scaled) per window
            kb_st = spool.tile([P, WPC * P], BF16)     # K per window
            vf_st = spool.tile([P, WPC * P], F32)      # V per window
            sel_all = spool.tile([P, TMAX * P], BF16)  # per-window one-hots
            msg_all = spool.tile([P, TMAX * P], F32)
            msgb_all = spool.tile([P, TMAX * P], BF16)
            expa_all = spool.tile([P, TMAX * HEADS], F32)
            expab_all = spool.tile([P, TMAX * HEADS], BF16)

            # ================= P1: QKV projection =================
            ps1_cm = tc.tile_pool(name="ps1", bufs=2, space="PSUM")
            pspool = ps1_cm.__enter__()
            for t in range(WPC):
                ht = wpool.tile([P, P], F16, tag="ht")
                nc.sync.dma_start(out=ht[:], in_=h_sl[t * P:(t + 1) * P, :])
                h32 = wpool.tile([P, P], F32, tag="h32")
                nc.vector.tensor_copy(out=h32[:], in_=ht[:])
                hb = wpool.tile([P, P], BF16, tag="hb")
                nc.vector.tensor_copy(out=hb[:], in_=h32[:])
                hr = wpool.tile([P, P], BF16, tag="hr")
                nc.vector.tensor_tensor(out=hr[:], in0=h32[:], in1=hb[:],
                                        op=mybir.AluOpType.subtract)
                hbT_ps = pspool.tile([P, P], BF16, tag="tp")
                nc.tensor.transpose(out=hbT_ps[:], in_=hb[:], identity=ident_b[:])
                hbT = wpool.tile([P, P], BF16, tag="hbT")
                nc.scalar.copy(out=hbT[:], in_=hbT_ps[:])
                hrT_ps = pspool.tile([P, P], BF16, tag="tpr")
                nc.tensor.transpose(out=hrT_ps[:], in_=hr[:], identity=ident_b[:])
                hrT = wpool.tile([P, P], BF16, tag="hrT")
                nc.scalar.copy(out=hrT[:], in_=hrT_ps[:])
                o_ps = pspool.tile([P, 3 * DIM], F32, tag="o")
                nc.tensor.matmul(out=o_ps[:], lhsT=hbT[:], rhs=wq_b[:],
                                 start=True, stop=False)
                nc.tensor.matmul(out=o_ps[:], lhsT=hrT[:], rhs=wq_b[:],
                                 start=False, stop=False)
                nc.tensor.matmul(out=o_ps[:], lhsT=hbT[:], rhs=wq_r[:],
                                 start=False, stop=True)
                qs = wpool.tile([P, DIM], F32, tag="qs")
                nc.scalar.copy(out=qs[:], in_=o_ps[:, 0:DIM])
                nc.sync.dma_start(out=qsl_d[t * P:(t + 1) * P, :], in_=qs[:])
                nc.vector.tensor_copy(out=qb_st[:, t * P:(t + 1) * P],
                                      in_=o_ps[:, 0:DIM])
                nc.vector.tensor_copy(out=kb_st[:, t * P:(t + 1) * P],
                                      in_=o_ps[:, DIM:2 * DIM])
                nc.vector.tensor_copy(out=vf_st[:, t * P:(t + 1) * P],
                                      in_=o_ps[:, 2 * DIM:3 * DIM])

            ps1_cm.__exit__(None, None, None)
            nc.gpsimd.collective_compute(
                "AllGather", mybir.AluOpType.bypass,
                replica_groups=[list(range(NCORES))],
                ins=[qsl_d[:].opt()], outs=[qfull[:].opt()])
            ps2_cm = tc.tile_pool(name="ps2", bufs=2, space="PSUM")
            pspool = ps2_cm.__enter__()
            acc2_cm = tc.tile_pool(name="acc2", bufs=1, space="PSUM")
            accpool = acc2_cm.__enter__()

            # ================= P2: denominators =================
            with tc.For_i(0, WPC, 1) as w:
                blk2 = wpool.tile([P, 3 * TMAX], U8, tag="blk2")
                nc.sync.dma_start(out=blk2[:],
                                  in_=idxpack2[:, ds(w * 3 * TMAX, 3 * TMAX)])
                lo32 = wpool.tile([P, TMAX], I32, tag="lo32")
                nc.vector.tensor_copy(out=lo32[:], in_=blk2[:, 0:TMAX])
                hi32 = wpool.tile([P, TMAX], I32, tag="hi32")
                nc.vector.tensor_copy(out=hi32[:], in_=blk2[:, TMAX:2 * TMAX])
                hs32 = wpool.tile([P, TMAX], I32, tag="hs32")
                nc.vector.tensor_scalar_mul(hs32[:], hi32[:], 256)
                iblk = wpool.tile([P, TMAX], I32, tag="iblk")
                nc.vector.tensor_tensor(out=iblk[:], in0=hs32[:], in1=lo32[:],
                                        op=mybir.AluOpType.add)
                jlf = wpool.tile([P, TMAX], F32, tag="jlf")
                nc.vector.tensor_copy(out=jlf[:], in_=blk2[:, 2 * TMAX:3 * TMAX])

                for t in range(TMAX):
                    qe = gpool.tile([P, DIM], F32, tag="qe")
                    nc.gpsimd.indirect_dma_start(
                        out=qe[:], out_offset=None, in_=qfull[:],
                        in_offset=bass.IndirectOffsetOnAxis(
                            ap=iblk[:, t:t + 1], axis=0))
                    nc.vector.tensor_tensor(
                        out=sel_all[:, t * P:(t + 1) * P],
                        in0=jlf[:, t:t + 1].to_broadcast([P, P]), in1=iotaPQ[:],
                        op=mybir.AluOpType.is_equal)
                    selT_ps = pspool.tile([P, P], BF16, tag="selT")
                    nc.tensor.transpose(out=selT_ps[:],
                                        in_=sel_all[:, t * P:(t + 1) * P],
                                        identity=ident_b[:])
                    selT = wpool.tile([P, P], BF16, tag="selTs")
                    nc.scalar.copy(out=selT[:], in_=selT_ps[:])
                    ke_ps = pspool.tile([P, DIM], F32, tag="ke")
                    nc.tensor.matmul(out=ke_ps[:], lhsT=selT[:],
                                     rhs=kb_st[:, ds(w * P, P)],
                                     start=True, stop=True)
                    prod = wpool.tile([P, DIM], F32, tag="prod")
                    nc.vector.tensor_tensor(out=prod[:], in0=qe[:], in1=ke_ps[:],
                                            op=mybir.AluOpType.mult)
                    a_t = wpool.tile([P, HEADS], F32, tag="a_t")
                    nc.vector.tensor_reduce(
                        out=a_t[:], in_=prod[:].rearrange("p (h d) -> p h d", h=HEADS),
                        axis=mybir.AxisListType.X, op=mybir.AluOpType.add)
                    nc.scalar.activation(
                        out=expa_all[:, t * HEADS:(t + 1) * HEADS], in_=a_t[:],
                        func=mybir.ActivationFunctionType.Exp,
                        bias=negc[:, 0:1], scale=1.0)

                nc.vector.tensor_copy(out=expab_all[:], in_=expa_all[:])
                den_ps = accpool.tile([P, HEADS], F32, tag="den")
                for t in range(TMAX):
                    nc.tensor.matmul(
                        out=den_ps[:], lhsT=sel_all[:, t * P:(t + 1) * P],
                        rhs=expab_all[:, t * HEADS:(t + 1) * HEADS],
                        start=(t == 0), stop=(t == TMAX - 1))
                den_s = wpool.tile([P, HEADS], F32, tag="den_s")
                nc.vector.tensor_scalar_add(den_s[:], den_ps[:], 1e-20)
                rec = wpool.tile([P, HEADS], F32, tag="rec")
                nc.vector.reciprocal(out=rec[:], in_=den_s[:])
                kf = wpool.tile([P, DIM], F32, tag="kf")
                nc.vector.tensor_copy(out=kf[:], in_=kb_st[:, ds(w * P, P)])
                nc.sync.dma_start(out=kvn_d[ds(w * P, P), 0:DIM], in_=kf[:])
                vn = wpool.tile([P, DIM], F32, tag="vn")
                for h in range(HEADS):
                    nc.vector.tensor_scalar_mul(
                        vn[:, h * HD:(h + 1) * HD],
                        vf_st[:, ds(w * P + h * HD, HD)], rec[:, h:h + 1])
                nc.sync.dma_start(out=kvn_d[ds(w * P, P), DIM:2 * DIM], in_=vn[:])

            acc2_cm.__exit__(None, None, None)
            ps2_cm.__exit__(None, None, None)
            nc.gpsimd.collective_compute(
                "AllGather", mybir.AluOpType.bypass,
                replica_groups=[list(range(NCORES))],
                ins=[kvn_d[:].opt()], outs=[kvnfull[:].opt()])
            ps3_cm = tc.tile_pool(name="ps3", bufs=2, space="PSUM")
            pspool = ps3_cm.__enter__()
            acc3_cm = tc.tile_pool(name="acc3", bufs=1, space="PSUM")
            accpool = acc3_cm.__enter__()

            # ================= P3: attention + epilogue =================
            def layer_norm(src, tag):
                mu = wpool.tile([P, 1], F32, tag=f"{tag}mu")
                nc.vector.tensor_reduce(out=mu[:], in_=src,
                                        axis=mybir.AxisListType.X,
                                        op=mybir.AluOpType.add)
                mus = wpool.tile([P, 1], F32, tag=f"{tag}mus")
                nc.vector.tensor_scalar_mul(mus[:], mu[:], 1.0 / DIM)
                cen = wpool.tile([P, DIM], F32, tag=f"{tag}cen")
                nc.vector.tensor_scalar(out=cen[:], in0=src, scalar1=mus[:, 0:1],
                                        scalar2=None, op0=mybir.AluOpType.subtract)
                sq = wpool.tile([P, DIM], F32, tag=f"{tag}sq")
                vs = wpool.tile([P, 1], F32, tag=f"{tag}vs")
                nc.scalar.activation(out=sq[:], in_=cen[:],
                                     func=mybir.ActivationFunctionType.Square,
                                     accum_out=vs[:])
                sd = wpool.tile([P, 1], F32, tag=f"{tag}sd")
                nc.scalar.activation(out=sd[:], in_=vs[:],
                                     func=mybir.ActivationFunctionType.Sqrt,
                                     scale=1.0 / DIM, bias=eps_t[:, 0:1])
                rstd = wpool.tile([P, 1], F32, tag=f"{tag}rstd")
                nc.vector.reciprocal(out=rstd[:], in_=sd[:])
                o = wpool.tile([P, DIM], F32, tag=f"{tag}o")
                nc.vector.tensor_scalar_mul(o[:], cen[:], rstd[:, 0:1])
                return o

            with tc.For_i(0, WPC, 1) as w:
                blk3 = wpool.tile([P, 3 * TMAX], U8, tag="blk3")
                nc.sync.dma_start(
                    out=blk3[:], in_=idxpack3[:, ds(w * 3 * TMAX, 3 * TMAX)])
                lo33 = wpool.tile([P, TMAX], I32, tag="lo33")
                nc.vector.tensor_copy(out=lo33[:], in_=blk3[:, 0:TMAX])
                hi33 = wpool.tile([P, TMAX], I32, tag="hi33")
                nc.vector.tensor_copy(out=hi33[:], in_=blk3[:, TMAX:2 * TMAX])
                hs33 = wpool.tile([P, TMAX], I32, tag="hs33")
                nc.vector.tensor_scalar_mul(hs33[:], hi33[:], 256)
                jblk = wpool.tile([P, TMAX], I32, tag="jblk")
                nc.vector.tensor_tensor(out=jblk[:], in0=hs33[:], in1=lo33[:],
                                        op=mybir.AluOpType.add)
                ilf = wpool.tile([P, TMAX], F32, tag="ilf")
                nc.vector.tensor_copy(out=ilf[:], in_=blk3[:, 2 * TMAX:3 * TMAX])

                for t in range(TMAX):
                    kve = gpool.tile([P, 2 * DIM], F32, tag="kve")
                    nc.gpsimd.indirect_dma_start(
                        out=kve[:], out_offset=None, in_=kvnfull[:],
                        in_offset=bass.IndirectOffsetOnAxis(
                            ap=jblk[:, t:t + 1], axis=0))
                    nc.vector.tensor_tensor(
                        out=sel_all[:, t * P:(t + 1) * P],
                        in0=ilf[:, t:t + 1].to_broadcast([P, P]), in1=iotaPQ[:],
                        op=mybir.AluOpType.is_equal)
                    selT_ps = pspool.tile([P, P], BF16, tag="selT")
                    nc.tensor.transpose(out=selT_ps[:],
                                        in_=sel_all[:, t * P:(t + 1) * P],
                                        identity=ident_b[:])
                    selT = wpool.tile([P, P], BF16, tag="selTs")
                    nc.scalar.copy(out=selT[:], in_=selT_ps[:])
                    qe_ps = pspool.tile([P, DIM], F32, tag="qeps")
                    nc.tensor.matmul(out=qe_ps[:], lhsT=selT[:],
                                     rhs=qb_st[:, ds(w * P, P)],
                                     start=True, stop=True)
                    prod = wpool.tile([P, DIM], F32, tag="prod3")
                    nc.vector.tensor_tensor(out=prod[:], in0=qe_ps[:],
                                            in1=kve[:, 0:DIM],
                                            op=mybir.AluOpType.mult)
                    a_t = wpool.tile([P, HEADS], F32, tag="a_t3")
                    nc.vector.tensor_reduce(
                        out=a_t[:], in_=prod[:].rearrange("p (h d) -> p h d", h=HEADS),
                        axis=mybir.AxisListType.X, op=mybir.AluOpType.add)
                    expa = wpool.tile([P, HEADS], F32, tag="expa3")
                    nc.scalar.activation(out=expa[:], in_=a_t[:],
                                         func=mybir.ActivationFunctionType.Exp,
                                         bias=negc[:, 0:1], scale=1.0)
                    for h in range(HEADS):
                        nc.vector.tensor_scalar_mul(
                            msg_all[:, t * P + h * HD:t * P + (h + 1) * HD],
                            kve[:, DIM + h * HD:DIM + (h + 1) * HD],
                            expa[:, h:h + 1])

                nc.vector.tensor_copy(out=msgb_all[:], in_=msg_all[:])
                attn_ps = accpool.tile([P, DIM], F32, tag="attn")
                for t in range(TMAX):
                    nc.tensor.matmul(
                        out=attn_ps[:], lhsT=sel_all[:, t * P:(t + 1) * P],
                        rhs=msgb_all[:, t * P:(t + 1) * P],
                        start=(t == 0), stop=(t == TMAX - 1))

                # epilogue: h = LN1(h_one + attn); out = LN2(h + silu(h @ wm))
                h16w = wpool.tile([P, P], F16, tag="h16w")
                nc.sync.dma_start(out=h16w[:], in_=h_sl[ds(w * P, P), :])
                h32w = wpool.tile([P, P], F32, tag="h32w")
                nc.vector.tensor_copy(out=h32w[:], in_=h16w[:])
                h0 = wpool.tile([P, DIM], F32, tag="h0")
                nc.vector.tensor_tensor(out=h0[:], in0=attn_ps[:], in1=h32w[:],
                                        op=mybir.AluOpType.add)
                ln1 = layer_norm(h0[:], "l1")
                lnb = wpool.tile([P, P], BF16, tag="lnb")
                nc.vector.tensor_copy(out=lnb[:], in_=ln1[:])
                lt_ps = accpool.tile([P, P], BF16, tag="lt")
                nc.tensor.transpose(out=lt_ps[:], in_=lnb[:], identity=ident_b[:])
                lt = wpool.tile([P, P], BF16, tag="lt_s")
                nc.scalar.copy(out=lt[:], in_=lt_ps[:])
                y_ps = accpool.tile([P, DIM], F32, tag="y")
                nc.tensor.matmul(out=y_ps[:], lhsT=lt[:], rhs=wm_b[:],
                                 start=True, stop=False)
                nc.tensor.matmul(out=y_ps[:], lhsT=lt[:], rhs=wm_r[:],
                                 start=False, stop=True)
                y = wpool.tile([P, DIM], F32, tag="ysb")
                nc.scalar.activation(out=y[:], in_=y_ps[:],
                                     func=mybir.ActivationFunctionType.Silu)
                h2 = wpool.tile([P, DIM], F32, tag="h2")
                nc.vector.tensor_tensor(out=h2[:], in0=ln1[:], in1=y[:],
                                        op=mybir.AluOpType.add)
                ln2 = layer_norm(h2[:], "l2")
                o16 = wpool.tile([P, DIM], F16, tag="o16")
                nc.vector.tensor_copy(out=o16[:], in_=ln2[:])
                nc.sync.dma_start(out=out[ds(w * P, P), :], in_=o16[:])
            acc3_cm.__exit__(None, None, None)
            ps3_cm.__exit__(None, None, None)
    nc.compile()
    return nc


_arange_cache = {}


def _build_phase(key_arr, other_arr, E, TMAX):
    """Group edges by 128-node window of key; pad windows to TMAX*128 slots.
    Returns (other, loc) as [NCORES*128, WPC*TMAX] uint16/uint8 arrays laid
    out so column w*TMAX+t, partition p holds edge slot t*128+p of window w."""
    wid16 = (np.asarray(key_arr) >> 7).astype(np.uint16)
    order = np.argsort(wid16, kind="stable")     # radix: groups by window
    wid = wid16[order].astype(np.int64)
    cnt = np.bincount(wid, minlength=NW)
    if cnt.max() > TMAX * P:
        raise _WindowOverflow(int(cnt.max()))
    starts = np.zeros(NW, np.int64)
    np.cumsum(cnt[:-1], out=starts[1:])
    if E not in _arange_cache:
        _arange_cache[E] = np.arange(E, dtype=np.int64)
    dest = wid * np.int64(TMAX * P) + (_arange_cache[E] - starts[wid])
    oth = np.zeros(NW * TMAX * P, np.uint16)
    loc = np.full(NW * TMAX * P, 255, np.uint8)
    oth[dest] = other_arr[order].astype(np.uint16)
    loc[dest] = (key_arr[order] & 127).astype(np.uint8)
    oth = oth.reshape(NCORES, WPC, TMAX, P).transpose(0, 3, 1, 2).reshape(
        NCORES * P, WPC * TMAX)
    loc = loc.reshape(NCORES, WPC, TMAX, P).transpose(0, 3, 1, 2).reshape(
        NCORES * P, WPC * TMAX)
    return np.ascontiguousarray(oth), np.ascontiguousarray(loc)


def _pack_phase(oth, loc, TMAX):
    """[NCORES*P, C] u16 + u8 -> [NCORES*P, 3C] u8 with per-window interleave
    [lo T | hi T | loc T]."""
    R, C = oth.shape
    lo = (oth & 0xFF).astype(np.uint8).reshape(R, WPC, TMAX)
    hi = (oth >> 8).astype(np.uint8).reshape(R, WPC, TMAX)
    lc = loc.reshape(R, WPC, TMAX)
    return np.stack([lo, hi, lc], axis=2).reshape(R, 3 * C)


class _WindowOverflow(RuntimeError):
    def __init__(self, count):
        super().__init__(f"window edge count {count} exceeds padded capacity")
        self.count = count


def _get_launcher(nc, key):
    """jit-compiled single-launch dispatcher. Unlike run_bass_via_pjrt it
    creates the donated output buffers on-device (nothing shipped for them)
    and caches the compiled executable for repeat calls."""
    if ("launcher", key) in _cache:
        return _cache[("launcher", key)]
    import jax
    import jax.numpy as jnp
    from jax.experimental.shard_map import shard_map
    from jax.sharding import Mesh, NamedSharding, PartitionSpec
    from concourse import bass2jax, mybir as _mybir

    bass2jax.install_neuronx_cc_hook()
    partition_name = nc.partition_id_tensor.name if nc.partition_id_tensor else None
    in_names, out_names, out_avals = [], [], []
    for alloc in nc.m.functions[0].allocations:
        if not isinstance(alloc, _mybir.MemoryLocationSet):
            continue
        name = alloc.memorylocations[0].name
        if alloc.kind == "ExternalInput":
            if name != partition_name:
                in_names.append(name)
        elif alloc.kind == "ExternalOutput":
            shape = tuple(alloc.tensor_shape)
            out_avals.append(jax.core.ShapedArray(shape, _mybir.dt.np(alloc.dtype)))
            out_names.append(name)
    n_params = len(in_names)
    all_names = in_names + out_names + ([partition_name] if partition_name else [])

    def _body(*args):
        operands = list(args)
        if partition_name is not None:
            operands.append(bass2jax.partition_id_tensor())
        outs = bass2jax._bass_exec_p.bind(
            *operands,
            out_avals=tuple(out_avals),
            in_names=tuple(all_names),
            out_names=tuple(out_names),
            lowering_input_output_aliases=(),
            sim_require_finite=True,
            sim_require_nnan=True,
            nc=nc,
        )
        return tuple(outs)

    devices = jax.devices()[:NCORES]
    mesh = Mesh(np.asarray(devices), ("core",))
    sharding = NamedSharding(mesh, PartitionSpec("core"))
    n_outs = len(out_avals)
    donate = tuple(range(n_params, n_params + n_outs))
    sharded = jax.jit(
        shard_map(_body, mesh=mesh,
                  in_specs=(PartitionSpec("core"),) * (n_params + n_outs),
                  out_specs=(PartitionSpec("core"),) * n_outs,
                  check_rep=False),
        donate_argnums=donate, keep_unused=True)

    def make_zeros():
        return [
            jax.jit(lambda a=a: jnp.zeros((NCORES * a.shape[0],) + a.shape[1:],
                                          a.dtype), out_shardings=sharding)()
            for a in out_avals
        ]

    launcher = dict(call=sharded, in_names=in_names, out_names=out_names,
                    make_zeros=make_zeros, sharding=sharding)
    _cache[("launcher", key)] = launcher
    return launcher


def kernel(**inputs):
    import os
    import threading
    import time

    import jax
    tlog = []
    _t0 = time.time()

    def _tick(label):
        tlog.append((label, time.time() - _t0))

    h_one = np.asarray(inputs["h_one"], np.float32)
    w_qkv = np.asarray(inputs["W_qkv"], np.float32)
    w_mlp = np.asarray(inputs["W_mlp"], np.float32)
    i_arr = np.asarray(inputs["e_e_i"]).astype(np.int64)
    j_arr = np.asarray(inputs["e_e_j"]).astype(np.int64)
    E = len(i_arr)

    tmax = _cache.get("tmax", DEFAULT_TMAX)

    # stage the index prep so phase-2 arrays upload while phase-3 prep runs
    prep = {}

    def _prep2():
        try:
            oth, loc = _build_phase(j_arr, i_arr, E, tmax)
            prep["pack2"] = _pack_phase(oth, loc, tmax)
        except _WindowOverflow as e:
            prep["overflow2"] = e.count

    def _prep3():
        try:
            oth, loc = _build_phase(i_arr, j_arr, E, tmax)
            prep["pack3"] = _pack_phase(oth, loc, tmax)
        except _WindowOverflow as e:
            prep["overflow3"] = e.count

    th2 = threading.Thread(target=_prep2)
    th2.start()
    _tick("thread started")

    h16 = h_one.astype(np.float16)
    wq_scaled = w_qkv.copy()
    wq_scaled[:, :DIM] *= np.float32(SCALE)
    wqm_rep = np.tile(np.concatenate([wq_scaled, w_mlp], axis=1), (NCORES, 1))

    _tick("casts done")
    if ("nc", tmax) not in _cache:
        _cache[("nc", tmax)] = _build(tmax)
    _tick("build done")
    L = _get_launcher(_cache[("nc", tmax)], tmax)
    sh = L["sharding"]
    _tick("launcher ready")

    # start big uploads while the index prep thread still runs
    dev = {"h_sl": jax.device_put(h16, sh), "wqm": jax.device_put(wqm_rep, sh)}
    _tick("h/w device_put issued")
    th2.join()
    th3 = threading.Thread(target=_prep3)
    th3.start()
    _tick("prep2 joined")
    if "pack2" in prep:
        dev["idxpack2"] = jax.device_put(prep["pack2"], sh)
    _tick("idxpack2 device_put issued")
    th3.join()
    _tick("prep3 joined")

    over = max(prep.get("overflow2", 0), prep.get("overflow3", 0))
    if over:
        # rare fallback: a window exceeds tmax*128 edges — rebuild the
        # program with enough headroom and redo the prep
        tmax = -(-over // P) + 2
        _cache["tmax"] = tmax
        if ("nc", tmax) not in _cache:
            _cache[("nc", tmax)] = _build(tmax)
        L = _get_launcher(_cache[("nc", tmax)], tmax)
        sh = L["sharding"]
        prep.clear()
        _prep2()
        _prep3()
        dev["idxpack2"] = jax.device_put(prep["pack2"], sh)

    dev["idxpack3"] = jax.device_put(prep["pack3"], sh)
    _tick("idxpack3 device_put issued")
    zeros = L["make_zeros"]()
    _tick("zeros made")

    out_arrs = L["call"](*[dev[n] for n in L["in_names"]], *zeros)
    _tick("call returned")
    out16 = np.asarray(out_arrs[0])
    _tick("output fetched")
    if os.environ.get("KERNEL_TIMING"):
        prev = 0.0
        for label, t in tlog:
            print(f"  [{t:6.2f}s +{t-prev:5.2f}] {label}", flush=True)
            prev = t
    return out16.astype(np.float32)


# revision 6
# speedup vs baseline: 1.2047x; 1.1042x over previous
"""Trainium2 Bass kernel for nn_Attention_41343355191713 (GNN message-passing
attention). Single SPMD launch on 8 cores:

  P1: QKV projection on each core's 8192-node slice (Q pre-scaled), K/V/Q
      stashed in SBUF, Q slice AllGathered to every core.
  P2: edges sorted by j (host), sharded so core c owns all edges whose j lands
      in its node range. Per 128-node window: gather Q[i] rows (indirect DMA),
      K[j] via one-hot matmul from the SBUF stash, A = Q.K per head,
      exp(A - 8), segment-sum into denom via one-hot matmul (softmax without
      max-subtraction: |A| <= ~7 for this distribution, and a constant shift
      cancels exactly). V normalized by denom, K|Vn AllGathered.
  P3: edges sorted by i; per destination window gather K|Vn[j] rows, recompute
      A, w = exp(A-8)*Vn, segment-sum into attn via one-hot matmul, then the
      fused epilogue (residual + LN + silu MLP + LN) and fp16 store.

Indices ship as uint16/uint8 (6B/edge), h_one and the output as fp16 — the
axon tunnel (~70MB/s) dominates cost, so bytes moved is the metric.
"""

import sys

sys.path.insert(0, "/opt/trn_rl_repo")

import math

import numpy as np

import concourse.bass as bass
import concourse.bacc as bacc
import concourse.mybir as mybir
import concourse.tile as tile
from concourse.bass import ds
from concourse.bass_utils import run_bass_kernel_spmd
from concourse.masks import make_identity

N = 65536
DIM = 128
HEADS = 4
HD = DIM // HEADS
SCALE = 1.0 / math.sqrt(HD)
LN_EPS = 1e-6
NCORES = 8
P = 128
SLICE = N // NCORES          # 8192 nodes per core
WPC = SLICE // P             # 64 windows per core
NW = N // P                  # 512 windows global
DEFAULT_TMAX = 34            # padded 128-edge tiles per window
ECONST = 8.0                 # constant shift inside exp
F32 = mybir.dt.float32
F16 = mybir.dt.float16
BF16 = mybir.dt.bfloat16
I32 = mybir.dt.int32
U16 = mybir.dt.uint16
U8 = mybir.dt.uint8

_cache = {}


def _build_impl(TMAX):
    nc = bacc.Bacc(None, target_bir_lowering=False, num_devices=NCORES,
                   disable_frame_to_traceback=True)
    h_sl = nc.declare_dram_parameter("h_sl", [SLICE, DIM], F16, isOutput=False)
    wqm = nc.declare_dram_parameter("wqm", [DIM, 4 * DIM], F32, isOutput=False)
    # index packs (u8): per-window blocks of 3T cols [lo T | hi T | loc T]
    idxpack2 = nc.declare_dram_parameter("idxpack2", [P, 3 * WPC * TMAX], U8,
                                         isOutput=False)
    idxpack3 = nc.declare_dram_parameter("idxpack3", [P, 3 * WPC * TMAX], U8,
                                         isOutput=False)
    out = nc.declare_dram_parameter("out", [SLICE, DIM], F16, isOutput=True)

    qsl_d = nc.dram_tensor("qsl_d", [SLICE, DIM], F32, kind="Internal")
    kvn_d = nc.dram_tensor("kvn_d", [SLICE, 2 * DIM], F32, kind="Internal")
    qfull = nc.dram_tensor("qfull", [N, DIM], F32, kind="Internal")
    kvnfull = nc.dram_tensor("kvnfull", [N, 2 * DIM], F32, kind="Internal")

    with tile.TileContext(nc) as tc:
        with (
            tc.tile_pool(name="const", bufs=1) as cpool,
            tc.tile_pool(name="stash", bufs=1) as spool,
            tc.tile_pool(name="work", bufs=3) as wpool,
            tc.tile_pool(name="gath", bufs=4) as gpool,
        ):
            # ---- constants ----
            ident = cpool.tile([P, P], F32)
            make_identity(nc, ident[:])
            ident_b = cpool.tile([P, P], BF16)
            nc.vector.tensor_copy(out=ident_b[:], in_=ident[:])
            iota_i = cpool.tile([P, P], I32)
            nc.gpsimd.iota(iota_i[:], pattern=[[1, P]], base=0, channel_multiplier=0)
            iotaPQ = cpool.tile([P, P], F32)
            nc.vector.tensor_copy(out=iotaPQ[:], in_=iota_i[:])
            negc = cpool.tile([P, 1], F32)
            nc.gpsimd.memset(negc[:], -ECONST)
            eps_t = cpool.tile([P, 1], F32)
            nc.gpsimd.memset(eps_t[:], LN_EPS)

            wq_f = cpool.tile([P, 3 * DIM], F32)
            nc.sync.dma_start(out=wq_f[:], in_=wqm[:, 0:3 * DIM])
            wq_b = cpool.tile([P, 3 * DIM], BF16)
            nc.vector.tensor_copy(out=wq_b[:], in_=wq_f[:])
            wq_r = cpool.tile([P, 3 * DIM], BF16)
            nc.vector.tensor_tensor(out=wq_r[:], in0=wq_f[:], in1=wq_b[:],
                                    op=mybir.AluOpType.subtract)
            wm_f = cpool.tile([P, DIM], F32)
            nc.sync.dma_start(out=wm_f[:], in_=wqm[:, 3 * DIM:4 * DIM])
            wm_b = cpool.tile([P, DIM], BF16)
            nc.vector.tensor_copy(out=wm_b[:], in_=wm_f[:])
            wm_r = cpool.tile([P, DIM], BF16)
            nc.vector.tensor_tensor(out=wm_r[:], in0=wm_f[:], in1=wm_b[:],
                                    op=mybir.AluOpType.subtract)

            # ---- persistent stashes ----
            qb_st = spool.tile([P, WPC * P], BF16)     # Q (scaled) per window
            kb_st = spool.tile([P, WPC * P], BF16)     # K per window
            vf_st = spool.tile([P, WPC * P], F32)      # V per window
            sel_all = spool.tile([P, TMAX * P], BF16)  # per-window one-hots
            msg_all = spool.tile([P, TMAX * P], F32)
            msgb_all = spool.tile([P, TMAX * P], BF16)
            expa_all = spool.tile([P, TMAX * HEADS], F32)
            expab_all = spool.tile([P, TMAX * HEADS], BF16)

            # ================= P1: QKV projection =================
            ps1_cm = tc.tile_pool(name="ps1", bufs=2, space="PSUM")
            pspool = ps1_cm.__enter__()
            for t in range(WPC):
                ht = wpool.tile([P, P], F16, tag="ht")
                nc.sync.dma_start(out=ht[:], in_=h_sl[t * P:(t + 1) * P, :])
                h32 = wpool.tile([P, P], F32, tag="h32")
                nc.vector.tensor_copy(out=h32[:], in_=ht[:])
                hb = wpool.tile([P, P], BF16, tag="hb")
                nc.vector.tensor_copy(out=hb[:], in_=h32[:])
                hr = wpool.tile([P, P], BF16, tag="hr")
                nc.vector.tensor_tensor(out=hr[:], in0=h32[:], in1=hb[:],
                                        op=mybir.AluOpType.subtract)
                hbT_ps = pspool.tile([P, P], BF16, tag="tp")
                nc.tensor.transpose(out=hbT_ps[:], in_=hb[:], identity=ident_b[:])
                hbT = wpool.tile([P, P], BF16, tag="hbT")
                nc.scalar.copy(out=hbT[:], in_=hbT_ps[:])
                hrT_ps = pspool.tile([P, P], BF16, tag="tpr")
                nc.tensor.transpose(out=hrT_ps[:], in_=hr[:], identity=ident_b[:])
                hrT = wpool.tile([P, P], BF16, tag="hrT")
                nc.scalar.copy(out=hrT[:], in_=hrT_ps[:])
                o_ps = pspool.tile([P, 3 * DIM], F32, tag="o")
                nc.tensor.matmul(out=o_ps[:], lhsT=hbT[:], rhs=wq_b[:],
                                 start=True, stop=False)
                nc.tensor.matmul(out=o_ps[:], lhsT=hrT[:], rhs=wq_b[:],
                                 start=False, stop=False)
                nc.tensor.matmul(out=o_ps[:], lhsT=hbT[:], rhs=wq_r[:],
                                 start=False, stop=True)
                qs = wpool.tile([P, DIM], F32, tag="qs")
                nc.scalar.copy(out=qs[:], in_=o_ps[:, 0:DIM])
                nc.sync.dma_start(out=qsl_d[t * P:(t + 1) * P, :], in_=qs[:])
                nc.vector.tensor_copy(out=qb_st[:, t * P:(t + 1) * P],
                                      in_=o_ps[:, 0:DIM])
                nc.vector.tensor_copy(out=kb_st[:, t * P:(t + 1) * P],
                                      in_=o_ps[:, DIM:2 * DIM])
                nc.vector.tensor_copy(out=vf_st[:, t * P:(t + 1) * P],
                                      in_=o_ps[:, 2 * DIM:3 * DIM])

            ps1_cm.__exit__(None, None, None)
            nc.gpsimd.collective_compute(
                "AllGather", mybir.AluOpType.bypass,
                replica_groups=[list(range(NCORES))],
                ins=[qsl_d[:].opt()], outs=[qfull[:].opt()])
            ps2_cm = tc.tile_pool(name="ps2", bufs=2, space="PSUM")
            pspool = ps2_cm.__enter__()
            acc2_cm = tc.tile_pool(name="acc2", bufs=1, space="PSUM")
            accpool = acc2_cm.__enter__()

            # ================= P2: denominators =================
            with tc.For_i(0, WPC, 1) as w:
                blk2 = wpool.tile([P, 3 * TMAX], U8, tag="blk2")
                nc.sync.dma_start(out=blk2[:],
                                  in_=idxpack2[:, ds(w * 3 * TMAX, 3 * TMAX)])
                lo32 = wpool.tile([P, TMAX], I32, tag="lo32")
                nc.vector.tensor_copy(out=lo32[:], in_=blk2[:, 0:TMAX])
                hi32 = wpool.tile([P, TMAX], I32, tag="hi32")
                nc.vector.tensor_copy(out=hi32[:], in_=blk2[:, TMAX:2 * TMAX])
                hs32 = wpool.tile([P, TMAX], I32, tag="hs32")
                nc.vector.tensor_scalar_mul(hs32[:], hi32[:], 256)
                iblk = wpool.tile([P, TMAX], I32, tag="iblk")
                nc.vector.tensor_tensor(out=iblk[:], in0=hs32[:], in1=lo32[:],
                                        op=mybir.AluOpType.add)
                jlf = wpool.tile([P, TMAX], F32, tag="jlf")
                nc.vector.tensor_copy(out=jlf[:], in_=blk2[:, 2 * TMAX:3 * TMAX])

                for t in range(TMAX):
                    qe = gpool.tile([P, DIM], F32, tag="qe")
                    nc.gpsimd.indirect_dma_start(
                        out=qe[:], out_offset=None, in_=qfull[:],
                        in_offset=bass.IndirectOffsetOnAxis(
                            ap=iblk[:, t:t + 1], axis=0))
                    nc.vector.tensor_tensor(
                        out=sel_all[:, t * P:(t + 1) * P],
                        in0=jlf[:, t:t + 1].to_broadcast([P, P]), in1=iotaPQ[:],
                        op=mybir.AluOpType.is_equal)
                    selT_ps = pspool.tile([P, P], BF16, tag="selT")
                    nc.tensor.transpose(out=selT_ps[:],
                                        in_=sel_all[:, t * P:(t + 1) * P],
                                        identity=ident_b[:])
                    selT = wpool.tile([P, P], BF16, tag="selTs")
                    nc.scalar.copy(out=selT[:], in_=selT_ps[:])
                    ke_ps = pspool.tile([P, DIM], F32, tag="ke")
                    nc.tensor.matmul(out=ke_ps[:], lhsT=selT[:],
                                     rhs=kb_st[:, ds(w * P, P)],
                                     start=True, stop=True)
                    prod = wpool.tile([P, DIM], F32, tag="prod")
                    nc.vector.tensor_tensor(out=prod[:], in0=qe[:], in1=ke_ps[:],
                                            op=mybir.AluOpType.mult)
                    a_t = wpool.tile([P, HEADS], F32, tag="a_t")
                    nc.vector.tensor_reduce(
                        out=a_t[:], in_=prod[:].rearrange("p (h d) -> p h d", h=HEADS),
                        axis=mybir.AxisListType.X, op=mybir.AluOpType.add)
                    nc.scalar.activation(
                        out=expa_all[:, t * HEADS:(t + 1) * HEADS], in_=a_t[:],
                        func=mybir.ActivationFunctionType.Exp,
                        bias=negc[:, 0:1], scale=1.0)

                nc.vector.tensor_copy(out=expab_all[:], in_=expa_all[:])
                den_ps = accpool.tile([P, HEADS], F32, tag="den")
                for t in range(TMAX):
                    nc.tensor.matmul(
                        out=den_ps[:], lhsT=sel_all[:, t * P:(t + 1) * P],
                        rhs=expab_all[:, t * HEADS:(t + 1) * HEADS],
                        start=(t == 0), stop=(t == TMAX - 1))
                den_s = wpool.tile([P, HEADS], F32, tag="den_s")
                nc.vector.tensor_scalar_add(den_s[:], den_ps[:], 1e-20)
                rec = wpool.tile([P, HEADS], F32, tag="rec")
                nc.vector.reciprocal(out=rec[:], in_=den_s[:])
                kf = wpool.tile([P, DIM], F32, tag="kf")
                nc.vector.tensor_copy(out=kf[:], in_=kb_st[:, ds(w * P, P)])
                nc.sync.dma_start(out=kvn_d[ds(w * P, P), 0:DIM], in_=kf[:])
                vn = wpool.tile([P, DIM], F32, tag="vn")
                for h in range(HEADS):
                    nc.vector.tensor_scalar_mul(
                        vn[:, h * HD:(h + 1) * HD],
                        vf_st[:, ds(w * P + h * HD, HD)], rec[:, h:h + 1])
                nc.sync.dma_start(out=kvn_d[ds(w * P, P), DIM:2 * DIM], in_=vn[:])

            acc2_cm.__exit__(None, None, None)
            ps2_cm.__exit__(None, None, None)
            nc.gpsimd.collective_compute(
                "AllGather", mybir.AluOpType.bypass,
                replica_groups=[list(range(NCORES))],
                ins=[kvn_d[:].opt()], outs=[kvnfull[:].opt()])
            ps3_cm = tc.tile_pool(name="ps3", bufs=2, space="PSUM")
            pspool = ps3_cm.__enter__()
            acc3_cm = tc.tile_pool(name="acc3", bufs=1, space="PSUM")
            accpool = acc3_cm.__enter__()

            # ================= P3: attention + epilogue =================
            def layer_norm(src, tag):
                mu = wpool.tile([P, 1], F32, tag=f"{tag}mu")
                nc.vector.tensor_reduce(out=mu[:], in_=src,
                                        axis=mybir.AxisListType.X,
                                        op=mybir.AluOpType.add)
                mus = wpool.tile([P, 1], F32, tag=f"{tag}mus")
                nc.vector.tensor_scalar_mul(mus[:], mu[:], 1.0 / DIM)
                cen = wpool.tile([P, DIM], F32, tag=f"{tag}cen")
                nc.vector.tensor_scalar(out=cen[:], in0=src, scalar1=mus[:, 0:1],
                                        scalar2=None, op0=mybir.AluOpType.subtract)
                sq = wpool.tile([P, DIM], F32, tag=f"{tag}sq")
                vs = wpool.tile([P, 1], F32, tag=f"{tag}vs")
                nc.scalar.activation(out=sq[:], in_=cen[:],
                                     func=mybir.ActivationFunctionType.Square,
                                     accum_out=vs[:])
                sd = wpool.tile([P, 1], F32, tag=f"{tag}sd")
                nc.scalar.activation(out=sd[:], in_=vs[:],
                                     func=mybir.ActivationFunctionType.Sqrt,
                                     scale=1.0 / DIM, bias=eps_t[:, 0:1])
                rstd = wpool.tile([P, 1], F32, tag=f"{tag}rstd")
                nc.vector.reciprocal(out=rstd[:], in_=sd[:])
                o = wpool.tile([P, DIM], F32, tag=f"{tag}o")
                nc.vector.tensor_scalar_mul(o[:], cen[:], rstd[:, 0:1])
                return o

            with tc.For_i(0, WPC, 1) as w:
                blk3 = wpool.tile([P, 3 * TMAX], U8, tag="blk3")
                nc.sync.dma_start(
                    out=blk3[:], in_=idxpack3[:, ds(w * 3 * TMAX, 3 * TMAX)])
                lo33 = wpool.tile([P, TMAX], I32, tag="lo33")
                nc.vector.tensor_copy(out=lo33[:], in_=blk3[:, 0:TMAX])
                hi33 = wpool.tile([P, TMAX], I32, tag="hi33")
                nc.vector.tensor_copy(out=hi33[:], in_=blk3[:, TMAX:2 * TMAX])
                hs33 = wpool.tile([P, TMAX], I32, tag="hs33")
                nc.vector.tensor_scalar_mul(hs33[:], hi33[:], 256)
                jblk = wpool.tile([P, TMAX], I32, tag="jblk")
                nc.vector.tensor_tensor(out=jblk[:], in0=hs33[:], in1=lo33[:],
                                        op=mybir.AluOpType.add)
                ilf = wpool.tile([P, TMAX], F32, tag="ilf")
                nc.vector.tensor_copy(out=ilf[:], in_=blk3[:, 2 * TMAX:3 * TMAX])

                for t in range(TMAX):
                    kve = gpool.tile([P, 2 * DIM], F32, tag="kve")
                    nc.gpsimd.indirect_dma_start(
                        out=kve[:], out_offset=None, in_=kvnfull[:],
                        in_offset=bass.IndirectOffsetOnAxis(
                            ap=jblk[:, t:t + 1], axis=0))
                    nc.vector.tensor_tensor(
                        out=sel_all[:, t * P:(t + 1) * P],
                        in0=ilf[:, t:t + 1].to_broadcast([P, P]), in1=iotaPQ[:],
                        op=mybir.AluOpType.is_equal)
                    selT_ps = pspool.tile([P, P], BF16, tag="selT")
                    nc.tensor.transpose(out=selT_ps[:],
                                        in_=sel_all[:, t * P:(t + 1) * P],
                                        identity=ident_b[:])
                    selT = wpool.tile([P, P], BF16, tag="selTs")
                    nc.scalar.copy(out=selT[:], in_=selT_ps[:])
                    qe_ps = pspool.tile([P, DIM], F32, tag="qeps")
                    nc.tensor.matmul(out=qe_ps[:], lhsT=selT[:],
                                     rhs=qb_st[:, ds(w * P, P)],
                                     start=True, stop=True)
                    prod = wpool.tile([P, DIM], F32, tag="prod3")
                    nc.vector.tensor_tensor(out=prod[:], in0=qe_ps[:],
                                            in1=kve[:, 0:DIM],
                                            op=mybir.AluOpType.mult)
                    a_t = wpool.tile([P, HEADS], F32, tag="a_t3")
                    nc.vector.tensor_reduce(
                        out=a_t[:], in_=prod[:].rearrange("p (h d) -> p h d", h=HEADS),
                        axis=mybir.AxisListType.X, op=mybir.AluOpType.add)
                    expa = wpool.tile([P, HEADS], F32, tag="expa3")
                    nc.scalar.activation(out=expa[:], in_=a_t[:],
                                         func=mybir.ActivationFunctionType.Exp,
                                         bias=negc[:, 0:1], scale=1.0)
                    for h in range(HEADS):
                        nc.vector.tensor_scalar_mul(
                            msg_all[:, t * P + h * HD:t * P + (h + 1) * HD],
                            kve[:, DIM + h * HD:DIM + (h + 1) * HD],
                            expa[:, h:h + 1])

                nc.vector.tensor_copy(out=msgb_all[:], in_=msg_all[:])
                attn_ps = accpool.tile([P, DIM], F32, tag="attn")
                for t in range(TMAX):
                    nc.tensor.matmul(
                        out=attn_ps[:], lhsT=sel_all[:, t * P:(t + 1) * P],
                        rhs=msgb_all[:, t * P:(t + 1) * P],
                        start=(t == 0), stop=(t == TMAX - 1))

                # epilogue: h = LN1(h_one + attn); out = LN2(h + silu(h @ wm))
                h16w = wpool.tile([P, P], F16, tag="h16w")
                nc.sync.dma_start(out=h16w[:], in_=h_sl[ds(w * P, P), :])
                h32w = wpool.tile([P, P], F32, tag="h32w")
                nc.vector.tensor_copy(out=h32w[:], in_=h16w[:])
                h0 = wpool.tile([P, DIM], F32, tag="h0")
                nc.vector.tensor_tensor(out=h0[:], in0=attn_ps[:], in1=h32w[:],
                                        op=mybir.AluOpType.add)
                ln1 = layer_norm(h0[:], "l1")
                lnb = wpool.tile([P, P], BF16, tag="lnb")
                nc.vector.tensor_copy(out=lnb[:], in_=ln1[:])
                lt_ps = accpool.tile([P, P], BF16, tag="lt")
                nc.tensor.transpose(out=lt_ps[:], in_=lnb[:], identity=ident_b[:])
                lt = wpool.tile([P, P], BF16, tag="lt_s")
                nc.scalar.copy(out=lt[:], in_=lt_ps[:])
                y_ps = accpool.tile([P, DIM], F32, tag="y")
                nc.tensor.matmul(out=y_ps[:], lhsT=lt[:], rhs=wm_b[:],
                                 start=True, stop=False)
                nc.tensor.matmul(out=y_ps[:], lhsT=lt[:], rhs=wm_r[:],
                                 start=False, stop=True)
                y = wpool.tile([P, DIM], F32, tag="ysb")
                nc.scalar.activation(out=y[:], in_=y_ps[:],
                                     func=mybir.ActivationFunctionType.Silu)
                h2 = wpool.tile([P, DIM], F32, tag="h2")
                nc.vector.tensor_tensor(out=h2[:], in0=ln1[:], in1=y[:],
                                        op=mybir.AluOpType.add)
                ln2 = layer_norm(h2[:], "l2")
                o16 = wpool.tile([P, DIM], F16, tag="o16")
                nc.vector.tensor_copy(out=o16[:], in_=ln2[:])
                nc.sync.dma_start(out=out[ds(w * P, P), :], in_=o16[:])
            acc3_cm.__exit__(None, None, None)
            ps3_cm.__exit__(None, None, None)
    nc.compile()
    return nc


_arange_cache = {}


def _build_phase(key_arr, other_arr, E, TMAX):
    """Group edges by 128-node window of key; pad windows to TMAX*128 slots.
    Returns (other, loc) as [NCORES*128, WPC*TMAX] uint16/uint8 arrays laid
    out so column w*TMAX+t, partition p holds edge slot t*128+p of window w."""
    wid16 = (np.asarray(key_arr) >> 7).astype(np.uint16)
    order = np.argsort(wid16, kind="stable")     # radix: groups by window
    wid = wid16[order].astype(np.int64)
    cnt = np.bincount(wid, minlength=NW)
    if cnt.max() > TMAX * P:
        raise _WindowOverflow(int(cnt.max()))
    starts = np.zeros(NW, np.int64)
    np.cumsum(cnt[:-1], out=starts[1:])
    if E not in _arange_cache:
        _arange_cache[E] = np.arange(E, dtype=np.int64)
    dest = wid * np.int64(TMAX * P) + (_arange_cache[E] - starts[wid])
    oth = np.zeros(NW * TMAX * P, np.uint16)
    loc = np.full(NW * TMAX * P, 255, np.uint8)
    oth[dest] = other_arr[order].astype(np.uint16)
    loc[dest] = (key_arr[order] & 127).astype(np.uint8)
    oth = oth.reshape(NCORES, WPC, TMAX, P).transpose(0, 3, 1, 2).reshape(
        NCORES * P, WPC * TMAX)
    loc = loc.reshape(NCORES, WPC, TMAX, P).transpose(0, 3, 1, 2).reshape(
        NCORES * P, WPC * TMAX)
    return np.ascontiguousarray(oth), np.ascontiguousarray(loc)


def _build(TMAX):
    """Run _build_impl with source frames bound to a stable synthetic filename
    and from a clean thread stack, so the emitted BIR (whose ant_debug embeds
    filenames/tracebacks) is byte-identical regardless of where kernel.py
    lives — keeping the on-disk NEFF cache valid across directories."""
    import inspect
    import threading

    if "_build_stable" not in _cache:
        ns = dict(globals())
        source = (inspect.getsource(_build_impl)
                  + "\n\ndef _thread_main(TMAX, out):\n"
                  + "    out.append(_build_impl(TMAX))\n")
        code = compile(source, "<gnn_attention_kernel>", "exec")
        exec(code, ns)
        _cache["_build_stable"] = ns["_thread_main"]
    result = []
    th = threading.Thread(target=_cache["_build_stable"], args=(TMAX, result))
    th.start()
    th.join()
    return result[0]


def _pack_phase(oth, loc, TMAX):
    """[NCORES*P, C] u16 + u8 -> [NCORES*P, 3C] u8 with per-window interleave
    [lo T | hi T | loc T]."""
    R, C = oth.shape
    lo = (oth & 0xFF).astype(np.uint8).reshape(R, WPC, TMAX)
    hi = (oth >> 8).astype(np.uint8).reshape(R, WPC, TMAX)
    lc = loc.reshape(R, WPC, TMAX)
    return np.stack([lo, hi, lc], axis=2).reshape(R, 3 * C)


class _WindowOverflow(RuntimeError):
    def __init__(self, count):
        super().__init__(f"window edge count {count} exceeds padded capacity")
        self.count = count


def _get_launcher(nc, key):
    """jit-compiled single-launch dispatcher. Unlike run_bass_via_pjrt it
    creates the donated output buffers on-device (nothing shipped for them)
    and caches the compiled executable for repeat calls."""
    if ("launcher", key) in _cache:
        return _cache[("launcher", key)]
    import jax
    import jax.numpy as jnp
    from jax.experimental.shard_map import shard_map
    from jax.sharding import Mesh, NamedSharding, PartitionSpec
    from concourse import bass2jax, mybir as _mybir

    bass2jax.install_neuronx_cc_hook()
    partition_name = nc.partition_id_tensor.name if nc.partition_id_tensor else None
    in_names, out_names, out_avals = [], [], []
    for alloc in nc.m.functions[0].allocations:
        if not isinstance(alloc, _mybir.MemoryLocationSet):
            continue
        name = alloc.memorylocations[0].name
        if alloc.kind == "ExternalInput":
            if name != partition_name:
                in_names.append(name)
        elif alloc.kind == "ExternalOutput":
            shape = tuple(alloc.tensor_shape)
            out_avals.append(jax.core.ShapedArray(shape, _mybir.dt.np(alloc.dtype)))
            out_names.append(name)
    n_params = len(in_names)
    all_names = in_names + out_names + ([partition_name] if partition_name else [])

    def _body(*args):
        operands = list(args)
        if partition_name is not None:
            operands.append(bass2jax.partition_id_tensor())
        outs = bass2jax._bass_exec_p.bind(
            *operands,
            out_avals=tuple(out_avals),
            in_names=tuple(all_names),
            out_names=tuple(out_names),
            lowering_input_output_aliases=(),
            sim_require_finite=True,
            sim_require_nnan=True,
            nc=nc,
        )
        return tuple(outs)

    devices = jax.devices()[:NCORES]
    mesh = Mesh(np.asarray(devices), ("core",))
    sharding = NamedSharding(mesh, PartitionSpec("core"))
    n_outs = len(out_avals)
    donate = tuple(range(n_params, n_params + n_outs))
    sharded = jax.jit(
        shard_map(_body, mesh=mesh,
                  in_specs=(PartitionSpec("core"),) * (n_params + n_outs),
                  out_specs=(PartitionSpec("core"),) * n_outs,
                  check_rep=False),
        donate_argnums=donate, keep_unused=True)

    def make_zeros():
        return [
            jax.jit(lambda a=a: jnp.zeros((NCORES * a.shape[0],) + a.shape[1:],
                                          a.dtype), out_shardings=sharding)()
            for a in out_avals
        ]

    launcher = dict(call=sharded, in_names=in_names, out_names=out_names,
                    make_zeros=make_zeros, sharding=sharding)
    _cache[("launcher", key)] = launcher
    return launcher


def kernel(**inputs):
    import os
    import threading
    import time

    import jax
    tlog = []
    _t0 = time.time()

    def _tick(label):
        tlog.append((label, time.time() - _t0))

    h_one = np.asarray(inputs["h_one"], np.float32)
    w_qkv = np.asarray(inputs["W_qkv"], np.float32)
    w_mlp = np.asarray(inputs["W_mlp"], np.float32)
    i_arr = np.asarray(inputs["e_e_i"]).astype(np.int64)
    j_arr = np.asarray(inputs["e_e_j"]).astype(np.int64)
    E = len(i_arr)

    tmax = _cache.get("tmax", DEFAULT_TMAX)

    # stage the index prep so phase-2 arrays upload while phase-3 prep runs
    prep = {}

    def _prep2():
        try:
            oth, loc = _build_phase(j_arr, i_arr, E, tmax)
            prep["pack2"] = _pack_phase(oth, loc, tmax)
        except _WindowOverflow as e:
            prep["overflow2"] = e.count

    def _prep3():
        try:
            oth, loc = _build_phase(i_arr, j_arr, E, tmax)
            prep["pack3"] = _pack_phase(oth, loc, tmax)
        except _WindowOverflow as e:
            prep["overflow3"] = e.count

    th2 = threading.Thread(target=_prep2)
    th2.start()
    _tick("thread started")

    h16 = h_one.astype(np.float16)
    wq_scaled = w_qkv.copy()
    wq_scaled[:, :DIM] *= np.float32(SCALE)
    wqm_rep = np.tile(np.concatenate([wq_scaled, w_mlp], axis=1), (NCORES, 1))

    _tick("casts done")
    if ("nc", tmax) not in _cache:
        _cache[("nc", tmax)] = _build(tmax)
    _tick("build done")
    L = _get_launcher(_cache[("nc", tmax)], tmax)
    sh = L["sharding"]
    _tick("launcher ready")

    # start big uploads while the index prep thread still runs
    dev = {"h_sl": jax.device_put(h16, sh), "wqm": jax.device_put(wqm_rep, sh)}
    _tick("h/w device_put issued")
    th2.join()
    th3 = threading.Thread(target=_prep3)
    th3.start()
    _tick("prep2 joined")
    if "pack2" in prep:
        dev["idxpack2"] = jax.device_put(prep["pack2"], sh)
    _tick("idxpack2 device_put issued")
    th3.join()
    _tick("prep3 joined")

    over = max(prep.get("overflow2", 0), prep.get("overflow3", 0))
    if over:
        # rare fallback: a window exceeds tmax*128 edges — rebuild the
        # program with enough headroom and redo the prep
        tmax = -(-over // P) + 2
        _cache["tmax"] = tmax
        if ("nc", tmax) not in _cache:
            _cache[("nc", tmax)] = _build(tmax)
        L = _get_launcher(_cache[("nc", tmax)], tmax)
        sh = L["sharding"]
        prep.clear()
        _prep2()
        _prep3()
        dev["idxpack2"] = jax.device_put(prep["pack2"], sh)

    dev["idxpack3"] = jax.device_put(prep["pack3"], sh)
    _tick("idxpack3 device_put issued")
    zeros = L["make_zeros"]()
    _tick("zeros made")

    out_arrs = L["call"](*[dev[n] for n in L["in_names"]], *zeros)
    _tick("call returned")
    out16 = np.asarray(out_arrs[0])
    _tick("output fetched")
    if os.environ.get("KERNEL_TIMING"):
        prev = 0.0
        for label, t in tlog:
            print(f"  [{t:6.2f}s +{t-prev:5.2f}] {label}", flush=True)
            prev = t
    return out16.astype(np.float32)


# revision 7
# speedup vs baseline: 1.2752x; 1.0585x over previous
"""Trainium2 Bass kernel for nn_Attention_41343355191713 (GNN message-passing
attention). Single SPMD launch on 8 cores:

  P1: QKV projection on each core's 8192-node slice (Q pre-scaled), K/V/Q
      stashed in SBUF, Q slice AllGathered to every core.
  P2: edges sorted by j (host), sharded so core c owns all edges whose j lands
      in its node range. Per 128-node window: gather Q[i] rows (indirect DMA),
      K[j] via one-hot matmul from the SBUF stash, A = Q.K per head,
      exp(A - 8), segment-sum into denom via one-hot matmul (softmax without
      max-subtraction: |A| <= ~7 for this distribution, and a constant shift
      cancels exactly). V normalized by denom, K|Vn AllGathered.
  P3: edges sorted by i; per destination window gather K|Vn[j] rows, recompute
      A, w = exp(A-8)*Vn, segment-sum into attn via one-hot matmul, then the
      fused epilogue (residual + LN + silu MLP + LN) and fp16 store.

Indices ship as uint16/uint8 (6B/edge), h_one and the output as fp16 — the
axon tunnel (~70MB/s) dominates cost, so bytes moved is the metric.
"""

import sys

sys.path.insert(0, "/opt/trn_rl_repo")

import math

import numpy as np

import concourse.bass as bass
import concourse.bacc as bacc
import concourse.mybir as mybir
import concourse.tile as tile
from concourse.bass import ds
from concourse.bass_utils import run_bass_kernel_spmd
from concourse.masks import make_identity

N = 65536
DIM = 128
HEADS = 4
HD = DIM // HEADS
SCALE = 1.0 / math.sqrt(HD)
LN_EPS = 1e-6
NCORES = 8
P = 128
SLICE = N // NCORES          # 8192 nodes per core
WPC = SLICE // P             # 64 windows per core
NW = N // P                  # 512 windows global
DEFAULT_TMAX = 34            # padded 128-edge tiles per window
ECONST = 8.0                 # constant shift inside exp
F32 = mybir.dt.float32
F16 = mybir.dt.float16
BF16 = mybir.dt.bfloat16
I32 = mybir.dt.int32
U16 = mybir.dt.uint16
U8 = mybir.dt.uint8

_cache = {}


def _build_impl(TMAX):
    nc = bacc.Bacc(None, target_bir_lowering=False, num_devices=NCORES,
                   disable_frame_to_traceback=True)
    h_sl = nc.declare_dram_parameter("h_sl", [SLICE, DIM], F16, isOutput=False)
    wqm = nc.declare_dram_parameter("wqm", [DIM, 4 * DIM], F32, isOutput=False)
    # index packs (u8): per-window blocks of 3T cols [lo T | hi T | loc T]
    idxpack2 = nc.declare_dram_parameter("idxpack2", [P, 3 * WPC * TMAX], U8,
                                         isOutput=False)
    idxpack3 = nc.declare_dram_parameter("idxpack3", [P, 3 * WPC * TMAX], U8,
                                         isOutput=False)
    out = nc.declare_dram_parameter("out", [SLICE, DIM], F16, isOutput=True)

    qsl_d = nc.dram_tensor("qsl_d", [SLICE, DIM], F32, kind="Internal")
    kvn_d = nc.dram_tensor("kvn_d", [SLICE, 2 * DIM], F32, kind="Internal")
    qfull = nc.dram_tensor("qfull", [N, DIM], F32, kind="Internal")
    kvnfull = nc.dram_tensor("kvnfull", [N, 2 * DIM], F32, kind="Internal")

    with tile.TileContext(nc) as tc:
        with (
            tc.tile_pool(name="const", bufs=1) as cpool,
            tc.tile_pool(name="stash", bufs=1) as spool,
            tc.tile_pool(name="work", bufs=3) as wpool,
            tc.tile_pool(name="gath", bufs=4) as gpool,
        ):
            # ---- constants ----
            ident = cpool.tile([P, P], F32)
            make_identity(nc, ident[:])
            ident_b = cpool.tile([P, P], BF16)
            nc.vector.tensor_copy(out=ident_b[:], in_=ident[:])
            iota_i = cpool.tile([P, P], I32)
            nc.gpsimd.iota(iota_i[:], pattern=[[1, P]], base=0, channel_multiplier=0)
            iotaPQ = cpool.tile([P, P], F32)
            nc.vector.tensor_copy(out=iotaPQ[:], in_=iota_i[:])
            negc = cpool.tile([P, 1], F32)
            nc.gpsimd.memset(negc[:], -ECONST)
            eps_t = cpool.tile([P, 1], F32)
            nc.gpsimd.memset(eps_t[:], LN_EPS)

            wq_f = cpool.tile([P, 3 * DIM], F32)
            nc.sync.dma_start(out=wq_f[:], in_=wqm[:, 0:3 * DIM])
            wq_b = cpool.tile([P, 3 * DIM], BF16)
            nc.vector.tensor_copy(out=wq_b[:], in_=wq_f[:])
            wq_r = cpool.tile([P, 3 * DIM], BF16)
            nc.vector.tensor_tensor(out=wq_r[:], in0=wq_f[:], in1=wq_b[:],
                                    op=mybir.AluOpType.subtract)
            wm_f = cpool.tile([P, DIM], F32)
            nc.sync.dma_start(out=wm_f[:], in_=wqm[:, 3 * DIM:4 * DIM])
            wm_b = cpool.tile([P, DIM], BF16)
            nc.vector.tensor_copy(out=wm_b[:], in_=wm_f[:])
            wm_r = cpool.tile([P, DIM], BF16)
            nc.vector.tensor_tensor(out=wm_r[:], in0=wm_f[:], in1=wm_b[:],
                                    op=mybir.AluOpType.subtract)

            # ---- persistent stashes ----
            qb_st = spool.tile([P, WPC * P], BF16)     # Q (scaled) per window
            kb_st = spool.tile([P, WPC * P], BF16)     # K per window
            vf_st = spool.tile([P, WPC * P], F32)      # V per window
            sel_all = spool.tile([P, TMAX * P], BF16)  # per-window one-hots
            msg_all = spool.tile([P, TMAX * P], F32)
            msgb_all = spool.tile([P, TMAX * P], BF16)
            expa_all = spool.tile([P, TMAX * HEADS], F32)
            expab_all = spool.tile([P, TMAX * HEADS], BF16)

            # ================= P1: QKV projection =================
            ps1_cm = tc.tile_pool(name="ps1", bufs=2, space="PSUM")
            pspool = ps1_cm.__enter__()
            for t in range(WPC):
                ht = wpool.tile([P, P], F16, tag="ht")
                nc.sync.dma_start(out=ht[:], in_=h_sl[t * P:(t + 1) * P, :])
                h32 = wpool.tile([P, P], F32, tag="h32")
                nc.vector.tensor_copy(out=h32[:], in_=ht[:])
                hb = wpool.tile([P, P], BF16, tag="hb")
                nc.vector.tensor_copy(out=hb[:], in_=h32[:])
                hr = wpool.tile([P, P], BF16, tag="hr")
                nc.vector.tensor_tensor(out=hr[:], in0=h32[:], in1=hb[:],
                                        op=mybir.AluOpType.subtract)
                hbT_ps = pspool.tile([P, P], BF16, tag="tp")
                nc.tensor.transpose(out=hbT_ps[:], in_=hb[:], identity=ident_b[:])
                hbT = wpool.tile([P, P], BF16, tag="hbT")
                nc.scalar.copy(out=hbT[:], in_=hbT_ps[:])
                hrT_ps = pspool.tile([P, P], BF16, tag="tpr")
                nc.tensor.transpose(out=hrT_ps[:], in_=hr[:], identity=ident_b[:])
                hrT = wpool.tile([P, P], BF16, tag="hrT")
                nc.scalar.copy(out=hrT[:], in_=hrT_ps[:])
                o_ps = pspool.tile([P, 3 * DIM], F32, tag="o")
                nc.tensor.matmul(out=o_ps[:], lhsT=hbT[:], rhs=wq_b[:],
                                 start=True, stop=False)
                nc.tensor.matmul(out=o_ps[:], lhsT=hrT[:], rhs=wq_b[:],
                                 start=False, stop=False)
                nc.tensor.matmul(out=o_ps[:], lhsT=hbT[:], rhs=wq_r[:],
                                 start=False, stop=True)
                qs = wpool.tile([P, DIM], F32, tag="qs")
                nc.scalar.copy(out=qs[:], in_=o_ps[:, 0:DIM])
                nc.sync.dma_start(out=qsl_d[t * P:(t + 1) * P, :], in_=qs[:])
                nc.vector.tensor_copy(out=qb_st[:, t * P:(t + 1) * P],
                                      in_=o_ps[:, 0:DIM])
                nc.vector.tensor_copy(out=kb_st[:, t * P:(t + 1) * P],
                                      in_=o_ps[:, DIM:2 * DIM])
                nc.vector.tensor_copy(out=vf_st[:, t * P:(t + 1) * P],
                                      in_=o_ps[:, 2 * DIM:3 * DIM])

            ps1_cm.__exit__(None, None, None)
            nc.gpsimd.collective_compute(
                "AllGather", mybir.AluOpType.bypass,
                replica_groups=[list(range(NCORES))],
                ins=[qsl_d[:].opt()], outs=[qfull[:].opt()])
            ps2_cm = tc.tile_pool(name="ps2", bufs=2, space="PSUM")
            pspool = ps2_cm.__enter__()
            acc2_cm = tc.tile_pool(name="acc2", bufs=1, space="PSUM")
            accpool = acc2_cm.__enter__()

            # ================= P2: denominators =================
            with tc.For_i(0, WPC, 1) as w:
                blk2 = wpool.tile([P, 3 * TMAX], U8, tag="blk2")
                nc.sync.dma_start(out=blk2[:],
                                  in_=idxpack2[:, ds(w * 3 * TMAX, 3 * TMAX)])
                lo32 = wpool.tile([P, TMAX], I32, tag="lo32")
                nc.vector.tensor_copy(out=lo32[:], in_=blk2[:, 0:TMAX])
                hi32 = wpool.tile([P, TMAX], I32, tag="hi32")
                nc.vector.tensor_copy(out=hi32[:], in_=blk2[:, TMAX:2 * TMAX])
                hs32 = wpool.tile([P, TMAX], I32, tag="hs32")
                nc.vector.tensor_scalar_mul(hs32[:], hi32[:], 256)
                iblk = wpool.tile([P, TMAX], I32, tag="iblk")
                nc.vector.tensor_tensor(out=iblk[:], in0=hs32[:], in1=lo32[:],
                                        op=mybir.AluOpType.add)
                jlf = wpool.tile([P, TMAX], F32, tag="jlf")
                nc.vector.tensor_copy(out=jlf[:], in_=blk2[:, 2 * TMAX:3 * TMAX])

                for t in range(TMAX):
                    qe = gpool.tile([P, DIM], F32, tag="qe")
                    nc.gpsimd.indirect_dma_start(
                        out=qe[:], out_offset=None, in_=qfull[:],
                        in_offset=bass.IndirectOffsetOnAxis(
                            ap=iblk[:, t:t + 1], axis=0))
                    nc.vector.tensor_tensor(
                        out=sel_all[:, t * P:(t + 1) * P],
                        in0=jlf[:, t:t + 1].to_broadcast([P, P]), in1=iotaPQ[:],
                        op=mybir.AluOpType.is_equal)
                    selT_ps = pspool.tile([P, P], BF16, tag="selT")
                    nc.tensor.transpose(out=selT_ps[:],
                                        in_=sel_all[:, t * P:(t + 1) * P],
                                        identity=ident_b[:])
                    selT = wpool.tile([P, P], BF16, tag="selTs")
                    nc.scalar.copy(out=selT[:], in_=selT_ps[:])
                    ke_ps = pspool.tile([P, DIM], F32, tag="ke")
                    nc.tensor.matmul(out=ke_ps[:], lhsT=selT[:],
                                     rhs=kb_st[:, ds(w * P, P)],
                                     start=True, stop=True)
                    prod = wpool.tile([P, DIM], F32, tag="prod")
                    nc.vector.tensor_tensor(out=prod[:], in0=qe[:], in1=ke_ps[:],
                                            op=mybir.AluOpType.mult)
                    a_t = wpool.tile([P, HEADS], F32, tag="a_t")
                    nc.vector.tensor_reduce(
                        out=a_t[:], in_=prod[:].rearrange("p (h d) -> p h d", h=HEADS),
                        axis=mybir.AxisListType.X, op=mybir.AluOpType.add)
                    nc.scalar.activation(
                        out=expa_all[:, t * HEADS:(t + 1) * HEADS], in_=a_t[:],
                        func=mybir.ActivationFunctionType.Exp,
                        bias=negc[:, 0:1], scale=1.0)

                nc.vector.tensor_copy(out=expab_all[:], in_=expa_all[:])
                den_ps = accpool.tile([P, HEADS], F32, tag="den")
                for t in range(TMAX):
                    nc.tensor.matmul(
                        out=den_ps[:], lhsT=sel_all[:, t * P:(t + 1) * P],
                        rhs=expab_all[:, t * HEADS:(t + 1) * HEADS],
                        start=(t == 0), stop=(t == TMAX - 1))
                den_s = wpool.tile([P, HEADS], F32, tag="den_s")
                nc.vector.tensor_scalar_add(den_s[:], den_ps[:], 1e-20)
                rec = wpool.tile([P, HEADS], F32, tag="rec")
                nc.vector.reciprocal(out=rec[:], in_=den_s[:])
                kf = wpool.tile([P, DIM], F32, tag="kf")
                nc.vector.tensor_copy(out=kf[:], in_=kb_st[:, ds(w * P, P)])
                nc.sync.dma_start(out=kvn_d[ds(w * P, P), 0:DIM], in_=kf[:])
                vn = wpool.tile([P, DIM], F32, tag="vn")
                for h in range(HEADS):
                    nc.vector.tensor_scalar_mul(
                        vn[:, h * HD:(h + 1) * HD],
                        vf_st[:, ds(w * P + h * HD, HD)], rec[:, h:h + 1])
                nc.sync.dma_start(out=kvn_d[ds(w * P, P), DIM:2 * DIM], in_=vn[:])

            acc2_cm.__exit__(None, None, None)
            ps2_cm.__exit__(None, None, None)
            nc.gpsimd.collective_compute(
                "AllGather", mybir.AluOpType.bypass,
                replica_groups=[list(range(NCORES))],
                ins=[kvn_d[:].opt()], outs=[kvnfull[:].opt()])
            ps3_cm = tc.tile_pool(name="ps3", bufs=2, space="PSUM")
            pspool = ps3_cm.__enter__()
            acc3_cm = tc.tile_pool(name="acc3", bufs=1, space="PSUM")
            accpool = acc3_cm.__enter__()

            # ================= P3: attention + epilogue =================
            def layer_norm(src, tag):
                mu = wpool.tile([P, 1], F32, tag=f"{tag}mu")
                nc.vector.tensor_reduce(out=mu[:], in_=src,
                                        axis=mybir.AxisListType.X,
                                        op=mybir.AluOpType.add)
                mus = wpool.tile([P, 1], F32, tag=f"{tag}mus")
                nc.vector.tensor_scalar_mul(mus[:], mu[:], 1.0 / DIM)
                cen = wpool.tile([P, DIM], F32, tag=f"{tag}cen")
                nc.vector.tensor_scalar(out=cen[:], in0=src, scalar1=mus[:, 0:1],
                                        scalar2=None, op0=mybir.AluOpType.subtract)
                sq = wpool.tile([P, DIM], F32, tag=f"{tag}sq")
                vs = wpool.tile([P, 1], F32, tag=f"{tag}vs")
                nc.scalar.activation(out=sq[:], in_=cen[:],
                                     func=mybir.ActivationFunctionType.Square,
                                     accum_out=vs[:])
                sd = wpool.tile([P, 1], F32, tag=f"{tag}sd")
                nc.scalar.activation(out=sd[:], in_=vs[:],
                                     func=mybir.ActivationFunctionType.Sqrt,
                                     scale=1.0 / DIM, bias=eps_t[:, 0:1])
                rstd = wpool.tile([P, 1], F32, tag=f"{tag}rstd")
                nc.vector.reciprocal(out=rstd[:], in_=sd[:])
                o = wpool.tile([P, DIM], F32, tag=f"{tag}o")
                nc.vector.tensor_scalar_mul(o[:], cen[:], rstd[:, 0:1])
                return o

            with tc.For_i(0, WPC, 1) as w:
                blk3 = wpool.tile([P, 3 * TMAX], U8, tag="blk3")
                nc.sync.dma_start(
                    out=blk3[:], in_=idxpack3[:, ds(w * 3 * TMAX, 3 * TMAX)])
                lo33 = wpool.tile([P, TMAX], I32, tag="lo33")
                nc.vector.tensor_copy(out=lo33[:], in_=blk3[:, 0:TMAX])
                hi33 = wpool.tile([P, TMAX], I32, tag="hi33")
                nc.vector.tensor_copy(out=hi33[:], in_=blk3[:, TMAX:2 * TMAX])
                hs33 = wpool.tile([P, TMAX], I32, tag="hs33")
                nc.vector.tensor_scalar_mul(hs33[:], hi33[:], 256)
                jblk = wpool.tile([P, TMAX], I32, tag="jblk")
                nc.vector.tensor_tensor(out=jblk[:], in0=hs33[:], in1=lo33[:],
                                        op=mybir.AluOpType.add)
                ilf = wpool.tile([P, TMAX], F32, tag="ilf")
                nc.vector.tensor_copy(out=ilf[:], in_=blk3[:, 2 * TMAX:3 * TMAX])

                for t in range(TMAX):
                    kve = gpool.tile([P, 2 * DIM], F32, tag="kve")
                    nc.gpsimd.indirect_dma_start(
                        out=kve[:], out_offset=None, in_=kvnfull[:],
                        in_offset=bass.IndirectOffsetOnAxis(
                            ap=jblk[:, t:t + 1], axis=0))
                    nc.vector.tensor_tensor(
                        out=sel_all[:, t * P:(t + 1) * P],
                        in0=ilf[:, t:t + 1].to_broadcast([P, P]), in1=iotaPQ[:],
                        op=mybir.AluOpType.is_equal)
                    selT_ps = pspool.tile([P, P], BF16, tag="selT")
                    nc.tensor.transpose(out=selT_ps[:],
                                        in_=sel_all[:, t * P:(t + 1) * P],
                                        identity=ident_b[:])
                    selT = wpool.tile([P, P], BF16, tag="selTs")
                    nc.scalar.copy(out=selT[:], in_=selT_ps[:])
                    qe_ps = pspool.tile([P, DIM], F32, tag="qeps")
                    nc.tensor.matmul(out=qe_ps[:], lhsT=selT[:],
                                     rhs=qb_st[:, ds(w * P, P)],
                                     start=True, stop=True)
                    prod = wpool.tile([P, DIM], F32, tag="prod3")
                    nc.vector.tensor_tensor(out=prod[:], in0=qe_ps[:],
                                            in1=kve[:, 0:DIM],
                                            op=mybir.AluOpType.mult)
                    a_t = wpool.tile([P, HEADS], F32, tag="a_t3")
                    nc.vector.tensor_reduce(
                        out=a_t[:], in_=prod[:].rearrange("p (h d) -> p h d", h=HEADS),
                        axis=mybir.AxisListType.X, op=mybir.AluOpType.add)
                    expa = wpool.tile([P, HEADS], F32, tag="expa3")
                    nc.scalar.activation(out=expa[:], in_=a_t[:],
                                         func=mybir.ActivationFunctionType.Exp,
                                         bias=negc[:, 0:1], scale=1.0)
                    for h in range(HEADS):
                        nc.vector.tensor_scalar_mul(
                            msg_all[:, t * P + h * HD:t * P + (h + 1) * HD],
                            kve[:, DIM + h * HD:DIM + (h + 1) * HD],
                            expa[:, h:h + 1])

                nc.vector.tensor_copy(out=msgb_all[:], in_=msg_all[:])
                attn_ps = accpool.tile([P, DIM], F32, tag="attn")
                for t in range(TMAX):
                    nc.tensor.matmul(
                        out=attn_ps[:], lhsT=sel_all[:, t * P:(t + 1) * P],
                        rhs=msgb_all[:, t * P:(t + 1) * P],
                        start=(t == 0), stop=(t == TMAX - 1))

                # epilogue: h = LN1(h_one + attn); out = LN2(h + silu(h @ wm))
                h16w = wpool.tile([P, P], F16, tag="h16w")
                nc.sync.dma_start(out=h16w[:], in_=h_sl[ds(w * P, P), :])
                h32w = wpool.tile([P, P], F32, tag="h32w")
                nc.vector.tensor_copy(out=h32w[:], in_=h16w[:])
                h0 = wpool.tile([P, DIM], F32, tag="h0")
                nc.vector.tensor_tensor(out=h0[:], in0=attn_ps[:], in1=h32w[:],
                                        op=mybir.AluOpType.add)
                ln1 = layer_norm(h0[:], "l1")
                lnb = wpool.tile([P, P], BF16, tag="lnb")
                nc.vector.tensor_copy(out=lnb[:], in_=ln1[:])
                lt_ps = accpool.tile([P, P], BF16, tag="lt")
                nc.tensor.transpose(out=lt_ps[:], in_=lnb[:], identity=ident_b[:])
                lt = wpool.tile([P, P], BF16, tag="lt_s")
                nc.scalar.copy(out=lt[:], in_=lt_ps[:])
                y_ps = accpool.tile([P, DIM], F32, tag="y")
                nc.tensor.matmul(out=y_ps[:], lhsT=lt[:], rhs=wm_b[:],
                                 start=True, stop=False)
                nc.tensor.matmul(out=y_ps[:], lhsT=lt[:], rhs=wm_r[:],
                                 start=False, stop=True)
                y = wpool.tile([P, DIM], F32, tag="ysb")
                nc.scalar.activation(out=y[:], in_=y_ps[:],
                                     func=mybir.ActivationFunctionType.Silu)
                h2 = wpool.tile([P, DIM], F32, tag="h2")
                nc.vector.tensor_tensor(out=h2[:], in0=ln1[:], in1=y[:],
                                        op=mybir.AluOpType.add)
                ln2 = layer_norm(h2[:], "l2")
                o16 = wpool.tile([P, DIM], F16, tag="o16")
                nc.vector.tensor_copy(out=o16[:], in_=ln2[:])
                nc.sync.dma_start(out=out[ds(w * P, P), :], in_=o16[:])
            acc3_cm.__exit__(None, None, None)
            ps3_cm.__exit__(None, None, None)
    nc.compile()
    return nc


_arange_cache = {}


def _build_phase(key_arr, other_arr, E, TMAX):
    """Group edges by 128-node window of key; pad windows to TMAX*128 slots.
    Returns (other, loc) as [NCORES*128, WPC*TMAX] uint16/uint8 arrays laid
    out so column w*TMAX+t, partition p holds edge slot t*128+p of window w."""
    wid16 = (np.asarray(key_arr) >> 7).astype(np.uint16)
    order = np.argsort(wid16, kind="stable")     # radix: groups by window
    wid = wid16[order].astype(np.int64)
    cnt = np.bincount(wid, minlength=NW)
    if cnt.max() > TMAX * P:
        raise _WindowOverflow(int(cnt.max()))
    starts = np.zeros(NW, np.int64)
    np.cumsum(cnt[:-1], out=starts[1:])
    if E not in _arange_cache:
        _arange_cache[E] = np.arange(E, dtype=np.int64)
    dest = wid * np.int64(TMAX * P) + (_arange_cache[E] - starts[wid])
    oth = np.zeros(NW * TMAX * P, np.uint16)
    loc = np.full(NW * TMAX * P, 255, np.uint8)
    oth[dest] = other_arr[order].astype(np.uint16)
    loc[dest] = (key_arr[order] & 127).astype(np.uint8)
    oth = oth.reshape(NCORES, WPC, TMAX, P).transpose(0, 3, 1, 2).reshape(
        NCORES * P, WPC * TMAX)
    loc = loc.reshape(NCORES, WPC, TMAX, P).transpose(0, 3, 1, 2).reshape(
        NCORES * P, WPC * TMAX)
    return np.ascontiguousarray(oth), np.ascontiguousarray(loc)


def _build(TMAX):
    """Run _build_impl with source frames bound to a stable synthetic filename
    and from a clean thread stack, so the emitted BIR (whose ant_debug embeds
    filenames/tracebacks) is byte-identical regardless of where kernel.py
    lives — keeping the on-disk NEFF cache valid across directories."""
    import inspect
    import threading

    if "_build_stable" not in _cache:
        ns = dict(globals())
        source = (inspect.getsource(_build_impl)
                  + "\n\ndef _thread_main(TMAX, out):\n"
                  + "    out.append(_build_impl(TMAX))\n")
        code = compile(source, "<gnn_attention_kernel>", "exec")
        exec(code, ns)
        _cache["_build_stable"] = ns["_thread_main"]
    result = []
    th = threading.Thread(target=_cache["_build_stable"], args=(TMAX, result))
    th.start()
    th.join()
    return result[0]


def _pack_phase(oth, loc, TMAX):
    """[NCORES*P, C] u16 + u8 -> [NCORES*P, 3C] u8 with per-window interleave
    [lo T | hi T | loc T]."""
    R, C = oth.shape
    lo = (oth & 0xFF).astype(np.uint8).reshape(R, WPC, TMAX)
    hi = (oth >> 8).astype(np.uint8).reshape(R, WPC, TMAX)
    lc = loc.reshape(R, WPC, TMAX)
    return np.stack([lo, hi, lc], axis=2).reshape(R, 3 * C)


class _WindowOverflow(RuntimeError):
    def __init__(self, count):
        super().__init__(f"window edge count {count} exceeds padded capacity")
        self.count = count


def _get_launcher(nc, key):
    """jit-compiled single-launch dispatcher. Unlike run_bass_via_pjrt it
    creates the donated output buffers on-device (nothing shipped for them)
    and caches the compiled executable for repeat calls."""
    if ("launcher", key) in _cache:
        return _cache[("launcher", key)]
    import jax
    import jax.numpy as jnp
    from jax.experimental.shard_map import shard_map
    from jax.sharding import Mesh, NamedSharding, PartitionSpec
    from concourse import bass2jax, mybir as _mybir

    bass2jax.install_neuronx_cc_hook()
    partition_name = nc.partition_id_tensor.name if nc.partition_id_tensor else None
    in_names, out_names, out_avals = [], [], []
    for alloc in nc.m.functions[0].allocations:
        if not isinstance(alloc, _mybir.MemoryLocationSet):
            continue
        name = alloc.memorylocations[0].name
        if alloc.kind == "ExternalInput":
            if name != partition_name:
                in_names.append(name)
        elif alloc.kind == "ExternalOutput":
            shape = tuple(alloc.tensor_shape)
            out_avals.append(jax.core.ShapedArray(shape, _mybir.dt.np(alloc.dtype)))
            out_names.append(name)
    n_params = len(in_names)
    all_names = in_names + out_names + ([partition_name] if partition_name else [])

    def _body(*args):
        operands = list(args)
        if partition_name is not None:
            operands.append(bass2jax.partition_id_tensor())
        outs = bass2jax._bass_exec_p.bind(
            *operands,
            out_avals=tuple(out_avals),
            in_names=tuple(all_names),
            out_names=tuple(out_names),
            lowering_input_output_aliases=(),
            sim_require_finite=True,
            sim_require_nnan=True,
            nc=nc,
        )
        return tuple(outs)

    devices = jax.devices()[:NCORES]
    mesh = Mesh(np.asarray(devices), ("core",))
    sharding = NamedSharding(mesh, PartitionSpec("core"))
    n_outs = len(out_avals)
    donate = tuple(range(n_params, n_params + n_outs))
    sharded = jax.jit(
        shard_map(_body, mesh=mesh,
                  in_specs=(PartitionSpec("core"),) * (n_params + n_outs),
                  out_specs=(PartitionSpec("core"),) * n_outs,
                  check_rep=False),
        donate_argnums=donate, keep_unused=True)

    def make_zeros():
        return [
            jax.jit(lambda a=a: jnp.zeros((NCORES * a.shape[0],) + a.shape[1:],
                                          a.dtype), out_shardings=sharding)()
            for a in out_avals
        ]

    launcher = dict(call=sharded, in_names=in_names, out_names=out_names,
                    make_zeros=make_zeros, sharding=sharding)
    _cache[("launcher", key)] = launcher
    return launcher


def kernel(**inputs):
    import os
    import threading
    import time

    import jax
    tlog = []
    _t0 = time.time()

    def _tick(label):
        tlog.append((label, time.time() - _t0))

    h_one = np.asarray(inputs["h_one"], np.float32)
    w_qkv = np.asarray(inputs["W_qkv"], np.float32)
    w_mlp = np.asarray(inputs["W_mlp"], np.float32)
    i_arr = np.asarray(inputs["e_e_i"]).astype(np.int64)
    j_arr = np.asarray(inputs["e_e_j"]).astype(np.int64)
    E = len(i_arr)

    tmax = _cache.get("tmax", DEFAULT_TMAX)

    # stage the index prep so phase-2 arrays upload while phase-3 prep runs
    prep = {}

    def _prep2():
        try:
            oth, loc = _build_phase(j_arr, i_arr, E, tmax)
            prep["pack2"] = _pack_phase(oth, loc, tmax)
        except _WindowOverflow as e:
            prep["overflow2"] = e.count

    def _prep3():
        try:
            oth, loc = _build_phase(i_arr, j_arr, E, tmax)
            prep["pack3"] = _pack_phase(oth, loc, tmax)
        except _WindowOverflow as e:
            prep["overflow3"] = e.count

    th2 = threading.Thread(target=_prep2)
    th2.start()
    _tick("thread started")

    h16 = h_one.astype(np.float16)
    _tick("casts done")
    if ("nc", tmax) not in _cache:
        _cache[("nc", tmax)] = _build(tmax)
    _tick("build done")
    L = _get_launcher(_cache[("nc", tmax)], tmax)
    sh = L["sharding"]
    _tick("launcher ready")

    # start big uploads while the index prep thread still runs; the donated
    # output buffers are input-independent, so create them on-device now too
    dev = {"h_sl": jax.device_put(h16, sh)}
    zeros = L["make_zeros"]()
    wq_scaled = w_qkv.copy()
    wq_scaled[:, :DIM] *= np.float32(SCALE)
    wqm_rep = np.tile(np.concatenate([wq_scaled, w_mlp], axis=1), (NCORES, 1))
    dev["wqm"] = jax.device_put(wqm_rep, sh)
    _tick("h/w device_put issued")
    th2.join()
    th3 = threading.Thread(target=_prep3)
    th3.start()
    _tick("prep2 joined")
    if "pack2" in prep:
        dev["idxpack2"] = jax.device_put(prep["pack2"], sh)
    _tick("idxpack2 device_put issued")
    th3.join()
    _tick("prep3 joined")

    over = max(prep.get("overflow2", 0), prep.get("overflow3", 0))
    if over:
        # rare fallback: a window exceeds tmax*128 edges — rebuild the
        # program with enough headroom and redo the prep
        tmax = -(-over // P) + 2
        _cache["tmax"] = tmax
        if ("nc", tmax) not in _cache:
            _cache[("nc", tmax)] = _build(tmax)
        L = _get_launcher(_cache[("nc", tmax)], tmax)
        sh = L["sharding"]
        zeros = L["make_zeros"]()
        dev["h_sl"] = jax.device_put(h16, sh)
        dev["wqm"] = jax.device_put(wqm_rep, sh)
        prep.clear()
        _prep2()
        _prep3()
        dev["idxpack2"] = jax.device_put(prep["pack2"], sh)

    dev["idxpack3"] = jax.device_put(prep["pack3"], sh)
    _tick("idxpack3 device_put issued")

    out_arrs = L["call"](*[dev[n] for n in L["in_names"]], *zeros)
    _tick("call returned")
    out16 = np.asarray(out_arrs[0])
    _tick("output fetched")
    if os.environ.get("KERNEL_TIMING"):
        prev = 0.0
        for label, t in tlog:
            print(f"  [{t:6.2f}s +{t-prev:5.2f}] {label}", flush=True)
            prev = t
    return out16.astype(np.float32)
